# revision 9
# baseline (speedup 1.0000x reference)
"""Trainium2 Bass kernel for nn_MoCA (self-attention + momentum concept attention).

Sharding: pure data parallel - batch dim (B=8) sharded 1 batch per NeuronCore,
weights/concept pool replicated. No collectives.

v5 - ACT(exp)-roofline-targeted redesign:
  * merged [wth|wph] conv (M=128, no duplication): one pass produces th on
    partitions 0:64 and ph on 64:128; ST runs as 32 sequential K=64 matmuls
    per n-block (quadrant pairing gave ~0 and required duplicated convs).
  * exp tiles widened to [128,1536] (3 m-chunks / 3 PSUM banks, bufs=2):
    11 ACT instructions per n-block instead of 16 -> less per-instr overhead.
  * g-conv and the MoCA M2@fm matmuls run in fp8e4 DoubleRow (2x PE rate);
    th/ph/score path stays bf16 for precision (fp8 there costs 1.7e-2 rel).
  * s2f is never materialized: MoCA scores accumulate fp8 M2@fm parts
    (emitted early to fill the SA->MoCA bubble) + bf16 m2b@lat parts in PSUM.
  * fm enters as host-cast bf16 (residual + th/ph conv) and fp8 (g conv,
    MoCA); output leaves as bf16 and is upcast on host. Halves the DMA.
  * norm path (PE transpose + reciprocal scale + DRAM reinterpret roundtrip)
    all in bf16.
  * PSUM: big [128,1536] x2 (6 banks) + pv x1 + tt x1 = 8 banks.
"""
import sys

if '/opt/trn_rl_repo' not in sys.path:
    sys.path.insert(0, '/opt/trn_rl_repo')

import numpy as np

C, L, H, W, P = 512, 64, 64, 64, 256
HW = H * W
B = 8
N_CORES = 8

_STATE: dict = {}


def _build_program(reps=1, num_devices=N_CORES, debug_dump=False):
    import concourse.bass as bass
    import concourse.bacc as bacc
    import concourse.mybir as mybir
    from concourse import tile
    from concourse.masks import make_identity

    dt = mybir.dt
    AFT = mybir.ActivationFunctionType
    DR = mybir.MatmulPerfMode.DoubleRow
    f32, bf16, f8 = dt.float32, dt.bfloat16, dt.float8e4

    nc = bacc.Bacc("TRN2", target_bir_lowering=False, debug=False,
                   enable_asserts=False, num_devices=num_devices)

    fmb_d = nc.dram_tensor("fmb", [C, HW], bf16, kind="ExternalInput").ap()
    fm8_d = nc.dram_tensor("fm8", [C, HW], f8, kind="ExternalInput").ap()
    wthph_d = nc.dram_tensor("wthph", [C, 128], bf16, kind="ExternalInput").ap()
    wg8_d = nc.dram_tensor("wg8", [256, 128], f8, kind="ExternalInput").ap()
    m2p_d = nc.dram_tensor("m2p", [256, 512], f8, kind="ExternalInput").ap()
    m2bt_d = nc.dram_tensor("m2bt", [L, P], bf16, kind="ExternalInput").ap()
    wcat_d = nc.dram_tensor("wcat", [128, C], bf16, kind="ExternalInput").ap()
    phiT_d = nc.dram_tensor("phiT", [P, L], bf16, kind="ExternalInput").ap()
    out_d = nc.dram_tensor("out", [C, HW], bf16, kind="ExternalOutput").ap()
    if debug_dump:
        dbg_th = nc.dram_tensor("dbg_th", [L, HW], bf16, kind="ExternalOutput").ap()
        dbg_ph = nc.dram_tensor("dbg_ph", [L, HW], bf16, kind="ExternalOutput").ap()
        dbg_g = nc.dram_tensor("dbg_g", [L, HW], bf16, kind="ExternalOutput").ap()
        dbg_lat = nc.dram_tensor("dbg_lat", [128, HW], bf16, kind="ExternalOutput").ap()
        dbg_at = nc.dram_tensor("dbg_at", [65, 512], bf16, kind="ExternalOutput").ap()

    NB = HW // 512          # 8 n-blocks of 512
    NM = HW // 128          # 32 m-chunks of 128
    NCC = C // 128          # 4 channel chunks
    # m-chunk groups per n-block: 10x3 + 1x2 (one exp instruction each)
    GROUPS = [tuple(range(3 * i, 3 * i + 3)) for i in range(10)] + [(30, 31)]
    BIGPAD = [128, 1536]

    with tile.TileContext(nc) as tc:
      for _rep in range(reps):
        with tc.tile_pool(name="sb", bufs=1) as sb, \
             tc.tile_pool(name="dram", bufs=1, space="DRAM") as dp, \
             tc.tile_pool(name="ps", bufs=1, space="PSUM") as psum:

            sc1 = dp.tile([HW, L], bf16, tag="sc1", name="sc1")
            sc2 = dp.tile([HW, L], bf16, tag="sc2", name="sc2")

            # ---------------- persistent tiles ----------------
            fmb = [[sb.tile([128, 512], bf16, tag=f"fmb{ci}_{nb}",
                            name=f"fmb{ci}_{nb}")
                    for nb in range(NB)] for ci in range(NCC)]
            fmf8 = [sb.tile([128, 2 * HW], f8, tag=f"fmf8{k}",
                            name=f"fmf8{k}") for k in range(2)]
            th_sb = sb.tile([64, HW], bf16, tag="th_sb", name="th_sb")
            ph_sb = sb.tile([64, HW], bf16, tag="ph_sb", name="ph_sb")
            g_sb = sb.tile([L, HW], bf16, tag="g_sb", name="g_sb")
            gto = sb.tile([128, NM * 65], bf16, tag="gto", name="gto")
            latcat = sb.tile([128, HW], bf16, tag="latcat", name="latcat")

            wthph_w = [sb.tile([128, 128], bf16, tag=f"wthph{i}",
                               name=f"wthph{i}") for i in range(NCC)]
            wg8_w = [sb.tile([128, 128], f8, tag=f"wg8{k}", name=f"wg8{k}")
                     for k in range(2)]
            m2p_w = [sb.tile([128, 512], f8, tag=f"m2p{k}", name=f"m2p{k}")
                     for k in range(2)]
            m2b_w = sb.tile([L, P], bf16, tag="m2b", name="m2b")
            wcat_w = sb.tile([128, C], bf16, tag="wcat", name="wcat")
            p2w = sb.tile([128, 130], bf16, tag="p2w", name="p2w")
            id64 = sb.tile([64, 64], bf16, tag="id64", name="id64")
            id65 = sb.tile([65, 65], bf16, tag="id65", name="id65")

            make_identity(nc, id64[:])
            make_identity(nc, id65[:])
            nc.vector.memset(p2w[:], 1.0)
            nc.vector.memset(gto[:], 1.0)

            # ---------------- weight loads (small, first) ----------------
            for ci in range(NCC):
                nc.sync.dma_start(wthph_w[ci][:],
                                  wthph_d[ci * 128:(ci + 1) * 128, :])
            for k in range(2):
                nc.sync.dma_start(wg8_w[k][:], wg8_d[k * 128:(k + 1) * 128, :])
                nc.sync.dma_start(m2p_w[k][:], m2p_d[k * 128:(k + 1) * 128, :])
            nc.sync.dma_start(m2b_w[:], m2bt_d)
            nc.sync.dma_start(wcat_w[:], wcat_d)
            nc.sync.dma_start(p2w[:, 0:64], phiT_d[0:128, :])
            nc.sync.dma_start(p2w[:, 65:129], phiT_d[128:256, :])

            # fp8 fm first (gates g conv + MoCA), then bf16 fm nb-by-nb
            for k in range(2):
                nc.sync.dma_start(fmf8[k][:, 0:HW],
                                  fm8_d[256 * k:256 * k + 128, :])
                nc.sync.dma_start(fmf8[k][:, HW:2 * HW],
                                  fm8_d[256 * k + 128:256 * k + 256, :])
            for nb in range(NB):
                ns = slice(nb * 512, (nb + 1) * 512)
                for ci in range(NCC):
                    nc.sync.dma_start(fmb[ci][nb][:],
                                      fmb_d[ci * 128:(ci + 1) * 128, ns])

            fmf8_v = [fmf8[k][:].rearrange("p (two n) -> p two n", two=2)
                      for k in range(2)]
            wg8_v = [wg8_w[k][:].rearrange("p (two m) -> p two m", two=2)
                     for k in range(2)]
            m2p_v = [m2p_w[k][:].rearrange("p (two m) -> p two m", two=2)
                     for k in range(2)]

            # -------- preamble convs: g (fp8 DR) + gT, then merged th|ph ----
            for nb in range(NB):
                ns = slice(nb * 512, (nb + 1) * 512)
                gp = psum.tile([L, 512], f32, tag="big", name="g_ps",
                               bufs=2, padded_shape=BIGPAD)
                for k in range(2):
                    nc.tensor.matmul(gp[:], wg8_v[k], fmf8_v[k][:, :, ns],
                                     start=(k == 0), stop=(k == 1),
                                     perf_mode=DR)
                nc.vector.tensor_copy(g_sb[:, ns], gp[:])
                for j in range(4):
                    mc = nb * 4 + j
                    tp = psum.tile([128, 64], bf16, tag="tt", name="gtp",
                                   bufs=1, padded_shape=[128, 512])
                    nc.tensor.transpose(tp[:], g_sb[:, mc * 128:(mc + 1) * 128],
                                        id64[:])
                    nc.vector.tensor_copy(gto[:, mc * 65:mc * 65 + 64], tp[:])
            def conv_thph(nb):
                ns2 = slice(nb * 512, (nb + 1) * 512)
                pp = psum.tile([128, 512], f32, tag="big", name="thph_ps",
                               bufs=2, padded_shape=BIGPAD)
                for ci in range(NCC):
                    nc.tensor.matmul(pp[:], wthph_w[ci][:], fmb[ci][nb][:],
                                     start=(ci == 0), stop=(ci == NCC - 1))
                nc.vector.tensor_copy(th_sb[:, ns2], pp[0:64, :])
                nc.vector.tensor_copy(ph_sb[:, ns2], pp[64:128, :])

            for nb in range(3):
                conv_thph(nb)
            fill_conv = [lambda b=b: conv_thph(b) for b in range(3, NB)]

            # ---------------- normalize helpers ----------------
            def norm_step(at, tbt, k):
                tp = psum.tile([128, 65], bf16, tag="tt", name="ntp",
                               bufs=1, padded_shape=[128, 512])
                nc.tensor.transpose(tp[:], at[:, k * 128:(k + 1) * 128],
                                    id65[:])
                rc = sb.tile([128, 1], f32, tag="rc", name="rc", bufs=2)
                nc.vector.reciprocal(rc[:], tp[:, 64:65])
                nc.vector.tensor_scalar_mul(tbt[:, k * 64:(k + 1) * 64],
                                            tp[:, 0:64], rc[:])

            def norm_dma(nbl, tbt, scr, dst_row0):
                scr_view = scr[nbl * 512:(nbl + 1) * 512, :].rearrange(
                    "(k p) c -> p k c", k=4)
                tbt_view = tbt[:].rearrange("p (k c) -> p k c", k=4)
                nc.sync.dma_start(scr_view, tbt_view)
                lat_view = scr[:].rearrange("(a b) c -> a (b c)", a=L)
                nc.sync.dma_start(latcat[dst_row0 + nbl * 8:
                                         dst_row0 + (nbl + 1) * 8, :],
                                  lat_view[nbl * 8:(nbl + 1) * 8, :])

            # ---------------- SA: ACT-paced pipeline ----------------
            sa_norm = {}
            for nb in range(NB):
                ns = slice(nb * 512, (nb + 1) * 512)
                pvp = psum.tile([65, 512], f32, tag="pv", name="pv", bufs=1,
                                padded_shape=[128, 512])
                prev = None
                for gi, grp in enumerate(GROUPS):
                    w = 512 * len(grp)
                    stp = psum.tile([128, w], f32, tag="big", name="st",
                                    bufs=2, padded_shape=BIGPAD)
                    for j, mc in enumerate(grp):
                        nc.tensor.matmul(
                            stp[:, j * 512:(j + 1) * 512],
                            ph_sb[:, mc * 128:(mc + 1) * 128],
                            th_sb[:, ns],
                            start=True, stop=True)
                    ptt = sb.tile([128, w], bf16, tag="pt", name="pt",
                                  bufs=3)
                    nc.scalar.activation(ptt[:], stp[:], AFT.Exp)
                    if nb == 0 and 1 <= gi <= 5 and fill_conv:
                        fill_conv.pop(0)()
                    if nb >= 1 and 1 <= gi <= 4:
                        norm_step(*sa_norm[nb - 1], gi - 1)
                    if nb >= 1 and gi == 5:
                        norm_dma(nb - 1, sa_norm.pop(nb - 1)[1], sc1, 0)
                    if prev is not None:
                        pgrp, pptt = prev
                        for j, mc in enumerate(pgrp):
                            nc.tensor.matmul(
                                pvp[:], gto[:, mc * 65:(mc + 1) * 65],
                                pptt[:, j * 512:(j + 1) * 512],
                                start=(mc == 0), stop=(mc == NM - 1))
                    prev = (grp, ptt)
                pgrp, pptt = prev
                for j, mc in enumerate(pgrp):
                    nc.tensor.matmul(pvp[:], gto[:, mc * 65:(mc + 1) * 65],
                                     pptt[:, j * 512:(j + 1) * 512],
                                     start=(mc == 0), stop=(mc == NM - 1))
                at = sb.tile([65, 512], bf16, tag="at", name="at", bufs=2)
                nc.vector.tensor_copy(at[:], pvp[:])
                if debug_dump and nb == 0:
                    nc.sync.dma_start(dbg_at, at[:])
                tbt = sb.tile([128, 256], bf16, tag="tb", name="tb", bufs=2)
                sa_norm[nb] = (at, tbt)
            # trailing SA norms interleave with early MoCA fm-score matmuls

            # ---------------- MoCA ----------------
            def s2_fm(nb):
                ns = slice(nb * 512, (nb + 1) * 512)
                s2p = psum.tile([128, 1024], f32, tag="big", name="s2",
                                bufs=2, padded_shape=BIGPAD)
                for k in range(2):
                    for pc in range(2):
                        nc.tensor.matmul(
                            s2p[:, pc * 512:(pc + 1) * 512],
                            m2p_v[k][:, :, pc * 128:(pc + 1) * 128],
                            fmf8_v[k][:, :, ns],
                            start=(k == 0), stop=False, perf_mode=DR)
                return s2p

            def s2_lat(nb, s2p):
                ns = slice(nb * 512, (nb + 1) * 512)
                for pc in range(2):
                    nc.tensor.matmul(s2p[:, pc * 512:(pc + 1) * 512],
                                     m2b_w[:, pc * 128:(pc + 1) * 128],
                                     latcat[0:64, ns],
                                     start=False, stop=True)
                p2t = sb.tile([128, 1024], bf16, tag="p2t", name="p2t",
                              bufs=3)
                nc.scalar.activation(p2t[:], s2p[:], AFT.Exp)
                return p2t

            def pv2_step(nb, p2t):
                pvq = psum.tile([65, 512], f32, tag="pv", name="pv2", bufs=1,
                                padded_shape=[128, 512])
                for pc in range(2):
                    nc.tensor.matmul(pvq[:], p2w[:, pc * 65:(pc + 1) * 65],
                                     p2t[:, pc * 512:(pc + 1) * 512],
                                     start=(pc == 0), stop=(pc == 1))
                at2 = sb.tile([65, 512], bf16, tag="at", name="at2", bufs=2)
                nc.vector.tensor_copy(at2[:], pvq[:])
                tb2 = sb.tile([128, 256], bf16, tag="tb", name="tb2", bufs=2)
                return (at2, tb2)

            s2ps = {0: s2_fm(0)}
            norm_step(*sa_norm[NB - 1], 0)
            s2ps[1] = s2_fm(1)
            for k in range(1, 4):
                norm_step(*sa_norm[NB - 1], k)
            norm_dma(NB - 1, sa_norm.pop(NB - 1)[1], sc1, 0)

            moca = {}
            mo_at = {}
            for nb in range(NB):
                if nb >= 2:
                    norm_step(*mo_at[nb - 2], 2)
                moca[nb] = s2_lat(nb, s2ps.pop(nb))
                if nb >= 2:
                    norm_step(*mo_at[nb - 2], 3)
                if nb >= 1:
                    mo_at[nb - 1] = pv2_step(nb - 1, moca.pop(nb - 1))
                if nb >= 2:
                    norm_dma(nb - 2, mo_at.pop(nb - 2)[1], sc2, 64)
                if nb + 2 < NB:
                    s2ps[nb + 2] = s2_fm(nb + 2)
                if nb >= 1:
                    norm_step(*mo_at[nb - 1], 0)
                    norm_step(*mo_at[nb - 1], 1)
            norm_step(*mo_at[NB - 2], 2)
            norm_step(*mo_at[NB - 2], 3)
            mo_at[NB - 1] = pv2_step(NB - 1, moca.pop(NB - 1))
            norm_dma(NB - 2, mo_at.pop(NB - 2)[1], sc2, 64)
            for k in range(4):
                norm_step(*mo_at[NB - 1], k)
            norm_dma(NB - 1, mo_at.pop(NB - 1)[1], sc2, 64)

            # ------------- tail: out = [wosa|womo]@[lat;lat2] + fm ----------
            for nb in range(NB):
                ns = slice(nb * 512, (nb + 1) * 512)
                for g2 in range(2):
                    oc = psum.tile([128, 1024], f32, tag="big", name="oc",
                                   bufs=2, padded_shape=BIGPAD)
                    ob = sb.tile([128, 1024], bf16, tag="ob", name="ob",
                                 bufs=3)
                    for h in range(2):
                        cc = g2 * 2 + h
                        nc.tensor.matmul(oc[:, h * 512:(h + 1) * 512],
                                         wcat_w[:, cc * 128:(cc + 1) * 128],
                                         latcat[:, ns],
                                         start=True, stop=True)
                        if h == 0:
                            nc.vector.tensor_add(
                                ob[:, h * 512:(h + 1) * 512],
                                oc[:, h * 512:(h + 1) * 512],
                                fmb[cc][nb][:])
                        else:
                            tmp = sb.tile([128, 512], bf16, tag="rtmp",
                                          name="rtmp", bufs=3)
                            nc.scalar.activation(tmp[:],
                                                 oc[:, h * 512:(h + 1) * 512],
                                                 AFT.Copy)
                            nc.gpsimd.tensor_add(
                                ob[:, h * 512:(h + 1) * 512], tmp[:],
                                fmb[cc][nb][:])
                    ov = out_d[g2 * 256:(g2 + 1) * 256, ns].rearrange(
                        "(u p) c -> p u c", u=2)
                    ob_view = ob[:].rearrange("p (u c) -> p u c", u=2)
                    nc.sync.dma_start(ov, ob_view)
            if debug_dump:
                nc.sync.dma_start(dbg_th, th_sb[:])
                nc.sync.dma_start(dbg_ph, ph_sb[:])
                nc.sync.dma_start(dbg_g, g_sb[:])
                nc.sync.dma_start(dbg_lat, latcat[:])

    nc.compile()
    return nc


def _get_runner(reps=1):
    """Build the Bass program once and return a cached jitted SPMD callable."""
    key = ("runner", reps)
    if key in _STATE:
        return _STATE[key]

    import jax
    import numpy as np
    from jax.experimental.shard_map import shard_map
    from jax.sharding import Mesh, PartitionSpec
    import concourse.mybir as mybir
    from concourse import bass2jax

    nc = _build_program(reps=reps)
    bass2jax.install_neuronx_cc_hook()

    partition_name = (nc.partition_id_tensor.name
                      if nc.partition_id_tensor else None)
    in_names, out_names, out_avals, zero_shapes = [], [], [], []
    for alloc in nc.m.functions[0].allocations:
        if not isinstance(alloc, mybir.MemoryLocationSet):
            continue
        name = alloc.memorylocations[0].name
        if alloc.kind == "ExternalInput":
            if name != partition_name:
                in_names.append(name)
        elif alloc.kind == "ExternalOutput":
            out_names.append(name)
            shape = tuple(alloc.tensor_shape)
            dtype = mybir.dt.np(alloc.dtype)
            out_avals.append(jax.core.ShapedArray(shape, dtype))
            zero_shapes.append((shape, dtype))
    n_params = len(in_names)
    all_in_names = list(in_names) + list(out_names)
    if partition_name is not None:
        all_in_names.append(partition_name)

    def _body(*args):
        operands = list(args)
        if partition_name is not None:
            operands.append(bass2jax.partition_id_tensor())
        outs = bass2jax._bass_exec_p.bind(
            *operands,
            out_avals=tuple(out_avals),
            in_names=tuple(all_in_names),
            out_names=tuple(out_names),
            lowering_input_output_aliases=(),
            sim_require_finite=True,
            sim_require_nnan=True,
            nc=nc,
        )
        return tuple(outs)

    devices = jax.devices()[:N_CORES]
    mesh = Mesh(np.asarray(devices), ("core",))
    n_outs = len(out_names)
    donate = tuple(range(n_params, n_params + n_outs))
    sharded = jax.jit(
        shard_map(_body, mesh=mesh,
                  in_specs=(PartitionSpec("core"),) * (n_params + n_outs),
                  out_specs=(PartitionSpec("core"),) * n_outs,
                  check_rep=False),
        donate_argnums=donate, keep_unused=True)

    runner = {
        "nc": nc, "sharded": sharded, "in_names": in_names,
        "out_names": out_names, "zero_shapes": zero_shapes,
        "n_params": n_params,
    }
    _STATE[key] = runner
    return runner


def _prep_in_maps(feature_map, concepts, w_theta, w_phi, w_g, w_o,
                  gamma_sa, gamma_moca):
    import ml_dtypes

    bf16 = ml_dtypes.bfloat16
    f8 = ml_dtypes.float8_e4m3fn

    feature_map = np.asarray(feature_map, dtype=np.float32)
    concepts = np.asarray(concepts, dtype=np.float32)
    w_theta = np.asarray(w_theta, dtype=np.float32)
    w_phi = np.asarray(w_phi, dtype=np.float32)
    w_g = np.asarray(w_g, dtype=np.float32)
    w_o = np.asarray(w_o, dtype=np.float32)
    gamma_sa = np.float32(gamma_sa)
    gamma_moca = np.float32(gamma_moca)

    gain = np.float32(1.0 / np.sqrt(C))
    gain_o = np.float32(1.0 / np.sqrt(L))

    wth_t = w_theta.T * gain                                 # [C, L]
    wph_t = w_phi.T * gain
    wthph = np.ascontiguousarray(
        np.concatenate([wth_t, wph_t], axis=1)).astype(bf16)  # [C, 128]
    wg_t = w_g.T * gain                                      # [C, L]
    wg8 = np.concatenate([
        np.concatenate([wg_t[256 * k:256 * k + 128],
                        wg_t[256 * k + 128:256 * k + 256]], axis=1)
        for k in range(2)], axis=0).astype(f8)               # [256, 128]
    m2 = concepts @ (w_theta * gain)                         # [P, C]
    m2t = m2.T                                               # [C, P]
    m2p = np.concatenate([
        np.concatenate([m2t[256 * k:256 * k + 128],
                        m2t[256 * k + 128:256 * k + 256]], axis=1)
        for k in range(2)], axis=0).astype(f8)               # [256, 512]
    m2b = (gamma_sa * gain_o) * (m2 @ w_o)                   # [P, L]
    m2bt = np.ascontiguousarray(m2b.T).astype(bf16)          # [L, P]
    wosa = w_o.T * (gain_o * gamma_sa)                       # [L, C]
    womo = w_o.T * (gain_o * gamma_moca)
    wcat = np.ascontiguousarray(
        np.concatenate([wosa, womo], axis=0)).astype(bf16)   # [128, C]
    phiT = np.ascontiguousarray(concepts).astype(bf16)       # [P, L]
    fm_flat = feature_map.reshape(B, C, HW)

    in_maps = []
    for b in range(N_CORES):
        fmb = np.ascontiguousarray(fm_flat[b]).astype(bf16)
        fm8 = np.ascontiguousarray(fm_flat[b]).astype(f8)
        in_maps.append({
            "fmb": fmb, "fm8": fm8,
            "wthph": wthph, "wg8": wg8, "m2p": m2p, "m2bt": m2bt,
            "wcat": wcat, "phiT": phiT,
        })
    return in_maps


def _run(in_maps):
    r = _get_runner()
    n_params = r["n_params"]
    concat_in = [
        np.concatenate([np.asarray(in_maps[c][name])
                        for c in range(N_CORES)], axis=0)
        for name in r["in_names"]
    ]
    concat_zeros = [np.zeros((N_CORES * s[0], *s[1:]), d)
                    for (s, d) in r["zero_shapes"]]
    out_arrs = r["sharded"](*concat_in, *concat_zeros)
    per_core = []
    for c in range(N_CORES):
        per_core.append({
            name: np.asarray(out_arrs[i]).reshape(
                N_CORES, *r["zero_shapes"][i][0])[c]
            for i, name in enumerate(r["out_names"])
        })
    return per_core


def kernel(feature_map, concepts, w_theta, w_phi, w_g, w_o,
           gamma_sa, gamma_moca):
    in_maps = _prep_in_maps(feature_map, concepts, w_theta, w_phi, w_g, w_o,
                            gamma_sa, gamma_moca)
    per_core = _run(in_maps)
    out = np.stack([per_core[b]["out"].astype(np.float32).reshape(C, H, W)
                    for b in range(B)], axis=0)
    return out.astype(np.float32)


# revision 10
# speedup vs baseline: 1.2039x; 1.2039x over previous
"""Trainium2 Bass kernel for nn_MoCA (self-attention + momentum concept attention).

Sharding: pure data parallel - batch dim (B=8) sharded 1 batch per NeuronCore,
weights/concept pool replicated. No collectives.

v6 - ACT(exp)-roofline-targeted redesign (all bf16 compute):
  * ST keeps the baseline quadrant pairing (dup'd th/ph convs, tile_position)
    -- required to sustain full PE clock -- but exp tiles are widened to
    [128,1536] (3 m-chunks: one pair + one single ST per group): 11 ACT
    instructions per n-block instead of 16.
  * all large DMAs are split into <=128KB chains: per-ring DMA bandwidth is
    ~23GB/s, so a single 512KB chain takes 22us. Splitting gets the first
    conv inputs on-chip in ~5us instead of 22us.
  * fm enters as host-cast bf16 (residual + convs); output leaves as bf16
    and is upcast on host. Halves the fm/out DMA vs f32.
  * conv work (ph/g for all blocks, th for block nb+1, M2@fm halves) is
    scheduled as fills inside the SA loop at >=3 exp-groups before first
    consumption (closer than that races the LDWEIGHTS prefetch).
  * MoCA scores: s2 = I@s2f + m2b@lat accumulated in PSUM; the identity
    matmuls are emitted before lat is ready to fill the SA->MoCA bubble.
  * norm path (PE transpose + reciprocal scale + DRAM reinterpret roundtrip)
    in bf16, spread across exp groups.
  * PSUM: big [128,1536] x2 (6 banks) + pv x1 + tt x1 = 8 banks.
"""
import sys

if '/opt/trn_rl_repo' not in sys.path:
    sys.path.insert(0, '/opt/trn_rl_repo')

import numpy as np

C, L, H, W, P = 512, 64, 64, 64, 256
HW = H * W
B = 8
N_CORES = 8

_STATE: dict = {}


def _build_program(reps=1, num_devices=N_CORES, debug_dump=False):
    import concourse.bass as bass
    import concourse.bacc as bacc
    import concourse.mybir as mybir
    from concourse import tile
    from concourse.masks import make_identity

    dt = mybir.dt
    AFT = mybir.ActivationFunctionType
    f32, bf16 = dt.float32, dt.bfloat16

    nc = bacc.Bacc("TRN2", target_bir_lowering=False, debug=False,
                   enable_asserts=False, num_devices=num_devices)

    fmb_d = nc.dram_tensor("fmb", [C, HW], bf16, kind="ExternalInput").ap()
    wthdup_d = nc.dram_tensor("wthdup", [C, 128], bf16, kind="ExternalInput").ap()
    wphdup_d = nc.dram_tensor("wphdup", [C, 128], bf16, kind="ExternalInput").ap()
    wg_d = nc.dram_tensor("wg", [C, L], bf16, kind="ExternalInput").ap()
    m2t_d = nc.dram_tensor("m2t", [C, P], bf16, kind="ExternalInput").ap()
    m2bt_d = nc.dram_tensor("m2bt", [L, P], bf16, kind="ExternalInput").ap()
    wcat_d = nc.dram_tensor("wcat", [128, C], bf16, kind="ExternalInput").ap()
    phiT_d = nc.dram_tensor("phiT", [P, L], bf16, kind="ExternalInput").ap()
    out_d = nc.dram_tensor("out", [C, HW], bf16, kind="ExternalOutput").ap()
    if debug_dump:
        dbg_th = nc.dram_tensor("dbg_th", [128, HW], bf16, kind="ExternalOutput").ap()
        dbg_ph = nc.dram_tensor("dbg_ph", [128, HW], bf16, kind="ExternalOutput").ap()
        dbg_g = nc.dram_tensor("dbg_g", [L, HW], bf16, kind="ExternalOutput").ap()
        dbg_lat = nc.dram_tensor("dbg_lat", [128, HW], bf16, kind="ExternalOutput").ap()
        dbg_at = nc.dram_tensor("dbg_at", [65, 512], bf16, kind="ExternalOutput").ap()

    NB = HW // 512          # 8 n-blocks of 512
    NM = HW // 128          # 32 m-chunks of 128
    NCC = C // 128          # 4 channel chunks
    # m-chunk groups per n-block: 10x3 + 1x2 (one exp instruction each)
    GROUPS = [tuple(range(3 * i, 3 * i + 3)) for i in range(10)] + [(30, 31)]
    BIGPAD = [128, 1536]

    with tile.TileContext(nc) as tc:
      for _rep in range(reps):
        with tc.tile_pool(name="sb", bufs=1) as sb, \
             tc.tile_pool(name="dram", bufs=1, space="DRAM") as dp, \
             tc.tile_pool(name="ps", bufs=1, space="PSUM") as psum:

            sc1 = dp.tile([HW, L], bf16, tag="sc1", name="sc1")
            sc2 = dp.tile([HW, L], bf16, tag="sc2", name="sc2")

            # ---------------- persistent tiles ----------------
            fmb = [[sb.tile([128, 512], bf16, tag=f"fmb{ci}_{nb}",
                            name=f"fmb{ci}_{nb}")
                    for nb in range(NB)] for ci in range(NCC)]
            th = sb.tile([128, HW], bf16, tag="th", name="th")
            ph = sb.tile([128, HW], bf16, tag="ph", name="ph")
            g_sb = sb.tile([L, HW], bf16, tag="g_sb", name="g_sb")
            gto = sb.tile([128, NM * 65], bf16, tag="gto", name="gto")
            s2f = [sb.tile([128, HW], bf16, tag=f"s2f{i}", name=f"s2f{i}")
                   for i in range(2)]
            latcat = sb.tile([128, HW], bf16, tag="latcat", name="latcat")

            wthd_w = [sb.tile([128, 128], bf16, tag=f"wthd{i}",
                              name=f"wthd{i}") for i in range(NCC)]
            wphd_w = [sb.tile([128, 128], bf16, tag=f"wphd{i}",
                              name=f"wphd{i}") for i in range(NCC)]
            wg_w = [sb.tile([128, L], bf16, tag=f"wg{i}", name=f"wg{i}")
                    for i in range(NCC)]
            m2_w = [[sb.tile([128, 128], bf16, tag=f"m2_{pc}_{i}",
                             name=f"m2_{pc}_{i}") for i in range(NCC)]
                    for pc in range(2)]
            m2b_w = sb.tile([L, P], bf16, tag="m2b", name="m2b")
            wcat_w = sb.tile([128, C], bf16, tag="wcat", name="wcat")
            p2w = sb.tile([128, 130], bf16, tag="p2w", name="p2w")
            id64 = sb.tile([64, 64], bf16, tag="id64", name="id64")
            id65 = sb.tile([65, 65], bf16, tag="id65", name="id65")
            id128 = sb.tile([128, 128], bf16, tag="id128", name="id128")

            make_identity(nc, id64[:])
            make_identity(nc, id65[:])
            make_identity(nc, id128[:])
            nc.vector.memset(p2w[:], 1.0)
            nc.vector.memset(gto[:], 1.0)

            # ---------------- weight loads (small, first) ----------------
            for ci in range(NCC):
                cs = slice(ci * 128, (ci + 1) * 128)
                nc.sync.dma_start(wthd_w[ci][:], wthdup_d[cs, :])
                nc.sync.dma_start(wphd_w[ci][:], wphdup_d[cs, :])
                nc.sync.dma_start(wg_w[ci][:], wg_d[cs, :])
                nc.sync.dma_start(m2_w[0][ci][:], m2t_d[cs, 0:128])
                nc.sync.dma_start(m2_w[1][ci][:], m2t_d[cs, 128:256])
            nc.sync.dma_start(m2b_w[:], m2bt_d)
            nc.sync.dma_start(wcat_w[:], wcat_d)
            nc.sync.dma_start(p2w[:, 0:64], phiT_d[0:128, :])
            nc.sync.dma_start(p2w[:, 65:129], phiT_d[128:256, :])

            # fm bf16, nb-major, 128KB chains (2 per [128,512] tile)
            for nb in range(NB):
                for ci in range(NCC):
                    for hf in range(2):
                        c0 = nb * 512 + hf * 256
                        nc.sync.dma_start(
                            fmb[ci][nb][:, hf * 256:(hf + 1) * 256],
                            fmb_d[ci * 128:(ci + 1) * 128, c0:c0 + 256])

            # ---------------- conv helpers ----------------
            def conv_th(nb):
                ns2 = slice(nb * 512, (nb + 1) * 512)
                pp = psum.tile([128, 512], f32, tag="big", name="th_ps",
                               bufs=2, padded_shape=BIGPAD)
                for ci in range(NCC):
                    nc.tensor.matmul(pp[:], wthd_w[ci][:], fmb[ci][nb][:],
                                     start=(ci == 0), stop=(ci == NCC - 1))
                nc.vector.tensor_copy(th[:, ns2], pp[:])

            def conv_ph(nb):
                ns2 = slice(nb * 512, (nb + 1) * 512)
                pp = psum.tile([128, 512], f32, tag="big", name="ph_ps",
                               bufs=2, padded_shape=BIGPAD)
                for ci in range(NCC):
                    nc.tensor.matmul(pp[:], wphd_w[ci][:], fmb[ci][nb][:],
                                     start=(ci == 0), stop=(ci == NCC - 1))
                nc.vector.tensor_copy(ph[:, ns2], pp[:])

            def conv_g(nb):
                ns2 = slice(nb * 512, (nb + 1) * 512)
                gp = psum.tile([L, 512], f32, tag="big", name="g_ps",
                               bufs=2, padded_shape=BIGPAD)
                for ci in range(NCC):
                    nc.tensor.matmul(gp[:], wg_w[ci][:], fmb[ci][nb][:],
                                     start=(ci == 0), stop=(ci == NCC - 1))
                nc.vector.tensor_copy(g_sb[:, ns2], gp[:])
                for j in range(4):
                    mc = nb * 4 + j
                    tp = psum.tile([128, 64], bf16, tag="tt", name="gtp",
                                   bufs=1, padded_shape=[128, 512])
                    nc.tensor.transpose(tp[:], g_sb[:, mc * 128:(mc + 1) * 128],
                                        id64[:])
                    nc.vector.tensor_copy(gto[:, mc * 65:mc * 65 + 64], tp[:])

            def conv_m2f(nb, pc):
                ns2 = slice(nb * 512, (nb + 1) * 512)
                mp = psum.tile([128, 512], f32, tag="big", name="m2_ps",
                               bufs=2, padded_shape=BIGPAD)
                for ci in range(NCC):
                    nc.tensor.matmul(mp[:], m2_w[pc][ci][:], fmb[ci][nb][:],
                                     start=(ci == 0), stop=(ci == NCC - 1))
                nc.vector.tensor_copy(s2f[pc][:, ns2], mp[:])

            # preamble: blocks 0..2 of th/ph/g; the rest are SA fills
            for nb in range(3):
                conv_th(nb)
                conv_ph(nb)
                conv_g(nb)

            # fills for SA(0): ph/g blocks 3..7, >=3 groups ahead of first use
            fills0 = {
                1: [lambda: conv_ph(3), lambda: conv_ph(4)],
                2: [lambda: conv_g(3), lambda: conv_ph(5)],
                3: [lambda: conv_g(4), lambda: conv_ph(6)],
                4: [lambda: conv_g(5), lambda: conv_ph(7)],
                5: [lambda: conv_g(6)],
                6: [lambda: conv_g(7)],
            }
            fill_m2f = [(b, pc) for b in range(NB) for pc in range(2)]

            # ---------------- normalize helpers ----------------
            def norm_step(at, tbt, k):
                tp = psum.tile([128, 65], bf16, tag="tt", name="ntp",
                               bufs=1, padded_shape=[128, 512])
                nc.tensor.transpose(tp[:], at[:, k * 128:(k + 1) * 128],
                                    id65[:])
                rc = sb.tile([128, 1], f32, tag="rc", name="rc", bufs=2)
                nc.vector.reciprocal(rc[:], tp[:, 64:65])
                nc.vector.tensor_scalar_mul(tbt[:, k * 64:(k + 1) * 64],
                                            tp[:, 0:64], rc[:])

            def norm_dma(nbl, tbt, scr, dst_row0):
                for hf in range(2):
                    scr_view = scr[nbl * 512 + hf * 256:
                                   nbl * 512 + (hf + 1) * 256, :].rearrange(
                        "(k p) c -> p k c", k=2)
                    tbt_view = tbt[:, hf * 128:(hf + 1) * 128].rearrange(
                        "p (k c) -> p k c", k=2)
                    nc.sync.dma_start(scr_view, tbt_view)
                lat_view = scr[:].rearrange("(a b) c -> a (b c)", a=L)
                for hf in range(2):
                    r0 = nbl * 8 + hf * 4
                    nc.sync.dma_start(
                        latcat[dst_row0 + r0:dst_row0 + r0 + 4, :],
                        lat_view[r0:r0 + 4, :])

            # ---------------- SA: ACT-paced pipeline ----------------
            sa_norm = {}
            for nb in range(NB):
                ns = slice(nb * 512, (nb + 1) * 512)
                pvp = psum.tile([65, 512], f32, tag="pv", name="pv", bufs=1,
                                padded_shape=[128, 512])
                prev = None
                for gi, grp in enumerate(GROUPS):
                    w = 512 * len(grp)
                    stp = psum.tile([128, w], f32, tag="big", name="st",
                                    bufs=2, padded_shape=BIGPAD)
                    # pair (h0,h64) + single (h0)
                    for j, mc in enumerate(grp[:2]):
                        hp = slice(64 * j, 64 * j + 64)
                        nc.tensor.matmul(
                            stp[:, j * 512:(j + 1) * 512],
                            ph[hp, mc * 128:(mc + 1) * 128],
                            th[hp, ns],
                            start=True, stop=True,
                            tile_position=(64 * j, 0))
                    if len(grp) == 3:
                        mc = grp[2]
                        nc.tensor.matmul(
                            stp[:, 1024:1536],
                            ph[0:64, mc * 128:(mc + 1) * 128],
                            th[0:64, ns],
                            start=True, stop=True,
                            tile_position=(0, 0))
                    ptt = sb.tile([128, w], bf16, tag="pt", name="pt",
                                  bufs=3)
                    nc.scalar.activation(ptt[:], stp[:], AFT.Exp)
                    if nb == 0:
                        for f in fills0.get(gi, []):
                            f()
                    else:
                        if 1 <= gi <= 4:
                            norm_step(*sa_norm[nb - 1], gi - 1)
                        if gi == 5:
                            norm_dma(nb - 1, sa_norm.pop(nb - 1)[1], sc1, 0)
                        if gi == 6 and nb < NB - 1:
                            conv_th(nb + 1)
                        if gi in (7, 9) and fill_m2f:
                            conv_m2f(*fill_m2f.pop(0))
                        if gi == 8 and nb >= 6 and fill_m2f:
                            conv_m2f(*fill_m2f.pop(0))
                    if prev is not None:
                        pgrp, pptt = prev
                        for j, mc in enumerate(pgrp):
                            nc.tensor.matmul(
                                pvp[:], gto[:, mc * 65:(mc + 1) * 65],
                                pptt[:, j * 512:(j + 1) * 512],
                                start=(mc == 0), stop=(mc == NM - 1))
                    prev = (grp, ptt)
                pgrp, pptt = prev
                for j, mc in enumerate(pgrp):
                    nc.tensor.matmul(pvp[:], gto[:, mc * 65:(mc + 1) * 65],
                                     pptt[:, j * 512:(j + 1) * 512],
                                     start=(mc == 0), stop=(mc == NM - 1))
                at = sb.tile([65, 512], bf16, tag="at", name="at", bufs=2)
                nc.vector.tensor_copy(at[:], pvp[:])
                if debug_dump and nb == 0:
                    nc.sync.dma_start(dbg_at, at[:])
                tbt = sb.tile([128, 256], bf16, tag="tb", name="tb", bufs=2)
                sa_norm[nb] = (at, tbt)

            # ---------------- MoCA ----------------
            def s2_fm(nb):
                ns = slice(nb * 512, (nb + 1) * 512)
                s2p = psum.tile([128, 1024], f32, tag="big", name="s2",
                                bufs=2, padded_shape=BIGPAD)
                for pc in range(2):
                    nc.tensor.matmul(s2p[:, pc * 512:(pc + 1) * 512],
                                     id128[:], s2f[pc][:, ns],
                                     start=True, stop=False)
                return s2p

            def s2_lat(nb, s2p):
                ns = slice(nb * 512, (nb + 1) * 512)
                for pc in range(2):
                    nc.tensor.matmul(s2p[:, pc * 512:(pc + 1) * 512],
                                     m2b_w[:, pc * 128:(pc + 1) * 128],
                                     latcat[0:64, ns],
                                     start=False, stop=True)
                p2t = sb.tile([128, 1024], bf16, tag="p2t", name="p2t",
                              bufs=3)
                nc.scalar.activation(p2t[:], s2p[:], AFT.Exp)
                return p2t

            def pv2_step(nb, p2t):
                pvq = psum.tile([65, 512], f32, tag="pv", name="pv2", bufs=1,
                                padded_shape=[128, 512])
                for pc in range(2):
                    nc.tensor.matmul(pvq[:], p2w[:, pc * 65:(pc + 1) * 65],
                                     p2t[:, pc * 512:(pc + 1) * 512],
                                     start=(pc == 0), stop=(pc == 1))
                at2 = sb.tile([65, 512], bf16, tag="at", name="at2", bufs=2)
                nc.vector.tensor_copy(at2[:], pvq[:])
                tb2 = sb.tile([128, 256], bf16, tag="tb", name="tb2", bufs=2)
                return (at2, tb2)

            # trailing SA norms interleaved with the MoCA fm-score prologue
            s2ps = {0: s2_fm(0)}
            norm_step(*sa_norm[NB - 1], 0)
            s2ps[1] = s2_fm(1)
            for k in range(1, 4):
                norm_step(*sa_norm[NB - 1], k)
            norm_dma(NB - 1, sa_norm.pop(NB - 1)[1], sc1, 0)

            moca = {}
            mo_at = {}
            for nb in range(NB):
                if nb >= 2:
                    norm_step(*mo_at[nb - 2], 2)
                moca[nb] = s2_lat(nb, s2ps.pop(nb))
                if nb >= 2:
                    norm_step(*mo_at[nb - 2], 3)
                if nb >= 1:
                    mo_at[nb - 1] = pv2_step(nb - 1, moca.pop(nb - 1))
                if nb >= 2:
                    norm_dma(nb - 2, mo_at.pop(nb - 2)[1], sc2, 64)
                if nb + 2 < NB:
                    s2ps[nb + 2] = s2_fm(nb + 2)
                if nb >= 1:
                    norm_step(*mo_at[nb - 1], 0)
                    norm_step(*mo_at[nb - 1], 1)
            norm_step(*mo_at[NB - 2], 2)
            norm_step(*mo_at[NB - 2], 3)
            mo_at[NB - 1] = pv2_step(NB - 1, moca.pop(NB - 1))
            norm_dma(NB - 2, mo_at.pop(NB - 2)[1], sc2, 64)
            for k in range(4):
                norm_step(*mo_at[NB - 1], k)
            norm_dma(NB - 1, mo_at.pop(NB - 1)[1], sc2, 64)

            # ------------- tail: out = [wosa|womo]@[lat;lat2] + fm ----------
            for nb in range(NB):
                ns = slice(nb * 512, (nb + 1) * 512)
                for g2 in range(2):
                    oc = psum.tile([128, 1024], f32, tag="big", name="oc",
                                   bufs=2, padded_shape=BIGPAD)
                    ob = sb.tile([128, 1024], bf16, tag="ob", name="ob",
                                 bufs=3)
                    for h in range(2):
                        cc = g2 * 2 + h
                        nc.tensor.matmul(oc[:, h * 512:(h + 1) * 512],
                                         wcat_w[:, cc * 128:(cc + 1) * 128],
                                         latcat[:, ns],
                                         start=True, stop=True)
                        if h == 0:
                            nc.vector.tensor_add(
                                ob[:, h * 512:(h + 1) * 512],
                                oc[:, h * 512:(h + 1) * 512],
                                fmb[cc][nb][:])
                        else:
                            tmp = sb.tile([128, 512], bf16, tag="rtmp",
                                          name="rtmp", bufs=3)
                            nc.scalar.activation(tmp[:],
                                                 oc[:, h * 512:(h + 1) * 512],
                                                 AFT.Copy)
                            nc.gpsimd.tensor_add(
                                ob[:, h * 512:(h + 1) * 512], tmp[:],
                                fmb[cc][nb][:])
                        nc.sync.dma_start(
                            out_d[g2 * 256 + h * 128:
                                  g2 * 256 + (h + 1) * 128, ns],
                            ob[:, h * 512:(h + 1) * 512])
            if debug_dump:
                nc.sync.dma_start(dbg_th, th[:])
                nc.sync.dma_start(dbg_ph, ph[:])
                nc.sync.dma_start(dbg_g, g_sb[:])
                nc.sync.dma_start(dbg_lat, latcat[:])

    nc.compile()
    return nc


def _get_runner(reps=1):
    """Build the Bass program once and return a cached jitted SPMD callable."""
    key = ("runner", reps)
    if key in _STATE:
        return _STATE[key]

    import jax
    import numpy as np
    from jax.experimental.shard_map import shard_map
    from jax.sharding import Mesh, PartitionSpec
    import concourse.mybir as mybir
    from concourse import bass2jax

    nc = _build_program(reps=reps)
    bass2jax.install_neuronx_cc_hook()

    partition_name = (nc.partition_id_tensor.name
                      if nc.partition_id_tensor else None)
    in_names, out_names, out_avals, zero_shapes = [], [], [], []
    for alloc in nc.m.functions[0].allocations:
        if not isinstance(alloc, mybir.MemoryLocationSet):
            continue
        name = alloc.memorylocations[0].name
        if alloc.kind == "ExternalInput":
            if name != partition_name:
                in_names.append(name)
        elif alloc.kind == "ExternalOutput":
            out_names.append(name)
            shape = tuple(alloc.tensor_shape)
            dtype = mybir.dt.np(alloc.dtype)
            out_avals.append(jax.core.ShapedArray(shape, dtype))
            zero_shapes.append((shape, dtype))
    n_params = len(in_names)
    all_in_names = list(in_names) + list(out_names)
    if partition_name is not None:
        all_in_names.append(partition_name)

    def _body(*args):
        operands = list(args)
        if partition_name is not None:
            operands.append(bass2jax.partition_id_tensor())
        outs = bass2jax._bass_exec_p.bind(
            *operands,
            out_avals=tuple(out_avals),
            in_names=tuple(all_in_names),
            out_names=tuple(out_names),
            lowering_input_output_aliases=(),
            sim_require_finite=True,
            sim_require_nnan=True,
            nc=nc,
        )
        return tuple(outs)

    devices = jax.devices()[:N_CORES]
    mesh = Mesh(np.asarray(devices), ("core",))
    n_outs = len(out_names)
    donate = tuple(range(n_params, n_params + n_outs))
    sharded = jax.jit(
        shard_map(_body, mesh=mesh,
                  in_specs=(PartitionSpec("core"),) * (n_params + n_outs),
                  out_specs=(PartitionSpec("core"),) * n_outs,
                  check_rep=False),
        donate_argnums=donate, keep_unused=True)

    runner = {
        "nc": nc, "sharded": sharded, "in_names": in_names,
        "out_names": out_names, "zero_shapes": zero_shapes,
        "n_params": n_params,
    }
    _STATE[key] = runner
    return runner


def _prep_in_maps(feature_map, concepts, w_theta, w_phi, w_g, w_o,
                  gamma_sa, gamma_moca):
    import ml_dtypes

    bf16 = ml_dtypes.bfloat16

    feature_map = np.asarray(feature_map, dtype=np.float32)
    concepts = np.asarray(concepts, dtype=np.float32)
    w_theta = np.asarray(w_theta, dtype=np.float32)
    w_phi = np.asarray(w_phi, dtype=np.float32)
    w_g = np.asarray(w_g, dtype=np.float32)
    w_o = np.asarray(w_o, dtype=np.float32)
    gamma_sa = np.float32(gamma_sa)
    gamma_moca = np.float32(gamma_moca)

    gain = np.float32(1.0 / np.sqrt(C))
    gain_o = np.float32(1.0 / np.sqrt(L))

    wth_t = w_theta.T * gain                                 # [C, L]
    wph_t = w_phi.T * gain
    wthdup = np.ascontiguousarray(
        np.concatenate([wth_t, wth_t], axis=1)).astype(bf16)
    wphdup = np.ascontiguousarray(
        np.concatenate([wph_t, wph_t], axis=1)).astype(bf16)
    wg_t = np.ascontiguousarray(w_g.T * gain).astype(bf16)   # [C, L]
    m2 = concepts @ (w_theta * gain)                         # [P, C]
    m2t = np.ascontiguousarray(m2.T).astype(bf16)            # [C, P]
    m2b = (gamma_sa * gain_o) * (m2 @ w_o)                   # [P, L]
    m2bt = np.ascontiguousarray(m2b.T).astype(bf16)          # [L, P]
    wosa = w_o.T * (gain_o * gamma_sa)                       # [L, C]
    womo = w_o.T * (gain_o * gamma_moca)
    wcat = np.ascontiguousarray(
        np.concatenate([wosa, womo], axis=0)).astype(bf16)   # [128, C]
    phiT = np.ascontiguousarray(concepts).astype(bf16)       # [P, L]
    fm_flat = feature_map.reshape(B, C, HW)

    in_maps = []
    for b in range(N_CORES):
        in_maps.append({
            "fmb": np.ascontiguousarray(fm_flat[b]).astype(bf16),
            "wthdup": wthdup, "wphdup": wphdup, "wg": wg_t,
            "m2t": m2t, "m2bt": m2bt, "wcat": wcat, "phiT": phiT,
        })
    return in_maps


def _run(in_maps):
    r = _get_runner()
    n_params = r["n_params"]
    concat_in = [
        np.concatenate([np.asarray(in_maps[c][name])
                        for c in range(N_CORES)], axis=0)
        for name in r["in_names"]
    ]
    concat_zeros = [np.zeros((N_CORES * s[0], *s[1:]), d)
                    for (s, d) in r["zero_shapes"]]
    out_arrs = r["sharded"](*concat_in, *concat_zeros)
    per_core = []
    for c in range(N_CORES):
        per_core.append({
            name: np.asarray(out_arrs[i]).reshape(
                N_CORES, *r["zero_shapes"][i][0])[c]
            for i, name in enumerate(r["out_names"])
        })
    return per_core


def kernel(feature_map, concepts, w_theta, w_phi, w_g, w_o,
           gamma_sa, gamma_moca):
    in_maps = _prep_in_maps(feature_map, concepts, w_theta, w_phi, w_g, w_o,
                            gamma_sa, gamma_moca)
    per_core = _run(in_maps)
    out = np.stack([per_core[b]["out"].astype(np.float32).reshape(C, H, W)
                    for b in range(B)], axis=0)
    return out.astype(np.float32)


# revision 12
# speedup vs baseline: 1.3180x; 1.0948x over previous
"""Trainium2 Bass kernel for nn_MoCA (self-attention + momentum concept attention).

Sharding: pure data parallel - batch dim (B=8) sharded 1 batch per NeuronCore,
weights/concept pool replicated. No collectives.

v6 - ACT(exp)-roofline-targeted redesign (all bf16 compute):
  * ST keeps the baseline quadrant pairing (dup'd th/ph convs, tile_position)
    -- required to sustain full PE clock -- but exp tiles are widened to
    [128,1536] (3 m-chunks: one pair + one single ST per group): 11 ACT
    instructions per n-block instead of 16.
  * all large DMAs are split into <=128KB chains: per-ring DMA bandwidth is
    ~23GB/s, so a single 512KB chain takes 22us. Splitting gets the first
    conv inputs on-chip in ~5us instead of 22us.
  * fm enters as host-cast bf16 (residual + convs); output leaves as bf16
    and is upcast on host. Halves the fm/out DMA vs f32.
  * conv work (ph/g for all blocks, th for block nb+1, M2@fm halves) is
    scheduled as fills inside the SA loop at >=3 exp-groups before first
    consumption (closer than that races the LDWEIGHTS prefetch).
  * MoCA scores: s2 = I@s2f + m2b@lat accumulated in PSUM; the identity
    matmuls are emitted before lat is ready to fill the SA->MoCA bubble.
  * norm path (PE transpose + reciprocal scale + DRAM reinterpret roundtrip)
    in bf16, spread across exp groups.
  * PSUM: big [128,1536] x2 (6 banks) + pv x1 + tt x1 = 8 banks.
"""
import sys

if '/opt/trn_rl_repo' not in sys.path:
    sys.path.insert(0, '/opt/trn_rl_repo')

import numpy as np

C, L, H, W, P = 512, 64, 64, 64, 256
HW = H * W
B = 8
N_CORES = 8

_STATE: dict = {}


def _build_program(reps=1, num_devices=N_CORES, debug_dump=False):
    import concourse.bass as bass
    import concourse.bacc as bacc
    import concourse.mybir as mybir
    from concourse import tile
    from concourse.masks import make_identity

    dt = mybir.dt
    AFT = mybir.ActivationFunctionType
    f32, bf16 = dt.float32, dt.bfloat16

    nc = bacc.Bacc("TRN2", target_bir_lowering=False, debug=False,
                   enable_asserts=False, num_devices=num_devices)

    fmb_d = nc.dram_tensor("fmb", [C, HW], bf16, kind="ExternalInput").ap()
    wall_d = nc.dram_tensor("wall", [128, 3200], bf16, kind="ExternalInput").ap()
    out_d = nc.dram_tensor("out", [C, HW], bf16, kind="ExternalOutput").ap()
    if debug_dump:
        dbg_th = nc.dram_tensor("dbg_th", [128, HW], bf16, kind="ExternalOutput").ap()
        dbg_ph = nc.dram_tensor("dbg_ph", [128, HW], bf16, kind="ExternalOutput").ap()
        dbg_g = nc.dram_tensor("dbg_g", [L, HW], bf16, kind="ExternalOutput").ap()
        dbg_lat = nc.dram_tensor("dbg_lat", [128, HW], bf16, kind="ExternalOutput").ap()
        dbg_at = nc.dram_tensor("dbg_at", [65, 512], bf16, kind="ExternalOutput").ap()

    NB = HW // 512          # 8 n-blocks of 512
    NM = HW // 128          # 32 m-chunks of 128
    NCC = C // 128          # 4 channel chunks
    # m-chunk groups per n-block: 10x3 + 1x2 (one exp instruction each)
    GROUPS = [tuple(range(3 * i, 3 * i + 3)) for i in range(10)] + [(30, 31)]
    BIGPAD = [128, 1536]

    with tile.TileContext(nc) as tc:
      for _rep in range(reps):
        with tc.tile_pool(name="sb", bufs=1) as sb, \
             tc.tile_pool(name="dram", bufs=1, space="DRAM") as dp, \
             tc.tile_pool(name="ps", bufs=1, space="PSUM") as psum:

            sc1 = dp.tile([HW, L], bf16, tag="sc1", name="sc1")
            sc2 = dp.tile([HW, L], bf16, tag="sc2", name="sc2")

            # ---------------- persistent tiles ----------------
            fmball = sb.tile([128, NCC * HW], bf16, tag="fmball",
                             name="fmball")
            fmb = [[fmball[:, ci * HW + nb * 512:ci * HW + (nb + 1) * 512]
                    for nb in range(NB)] for ci in range(NCC)]
            th = sb.tile([128, HW], bf16, tag="th", name="th")
            ph = sb.tile([128, HW], bf16, tag="ph", name="ph")
            g_sb = sb.tile([L, HW], bf16, tag="g_sb", name="g_sb")
            gto = sb.tile([128, NM * 65], bf16, tag="gto", name="gto")
            s2f = [sb.tile([128, HW], bf16, tag=f"s2f{i}", name=f"s2f{i}")
                   for i in range(2)]
            latcat = sb.tile([128, HW], bf16, tag="latcat", name="latcat")

            wall_w = sb.tile([128, 3200], bf16, tag="wall", name="wall")
            wthd_w = [wall_w[:, i * 128:(i + 1) * 128] for i in range(NCC)]
            wphd_w = [wall_w[:, 512 + i * 128:512 + (i + 1) * 128]
                      for i in range(NCC)]
            wg_w = [wall_w[:, 1024 + i * 64:1024 + (i + 1) * 64]
                    for i in range(NCC)]
            m2_w = [[wall_w[:, 1280 + pc * 512 + i * 128:
                            1280 + pc * 512 + (i + 1) * 128]
                     for i in range(NCC)] for pc in range(2)]
            m2b_w = wall_w[0:64, 2304:2560]
            wcat_w = wall_w[:, 2560:3072]
            p2w = sb.tile([128, 130], bf16, tag="p2w", name="p2w")
            id64 = sb.tile([64, 64], bf16, tag="id64", name="id64")
            id65 = sb.tile([65, 65], bf16, tag="id65", name="id65")
            id128 = sb.tile([128, 128], bf16, tag="id128", name="id128")

            make_identity(nc, id64[:])
            make_identity(nc, id65[:])
            make_identity(nc, id128[:])
            nc.vector.memset(p2w[:], 1.0)
            nc.vector.memset(gto[:], 1.0)

            # one DMA for all weights; 8 block-DMAs for fm (hardware
            # packetizes each instruction across 16 engines at 256B)
            nc.sync.dma_start(wall_w[:], wall_d)
            fm_dst = fmball[:].rearrange("p (ci n) -> p ci n", ci=NCC)
            fm_src = fmb_d[:].rearrange("(ci p) n -> p ci n", ci=NCC)
            for nb in range(NB):
                ns = slice(nb * 512, (nb + 1) * 512)
                nc.sync.dma_start(fm_dst[:, :, ns], fm_src[:, :, ns])
            nc.vector.tensor_copy(p2w[:, 0:64], wall_w[:, 3072:3136])
            nc.vector.tensor_copy(p2w[:, 65:129], wall_w[:, 3136:3200])

            # ---------------- conv helpers ----------------
            def conv_th(nb):
                ns2 = slice(nb * 512, (nb + 1) * 512)
                pp = psum.tile([128, 512], f32, tag="big", name="th_ps",
                               bufs=2, padded_shape=BIGPAD)
                for ci in range(NCC):
                    nc.tensor.matmul(pp[:], wthd_w[ci], fmb[ci][nb],
                                     start=(ci == 0), stop=(ci == NCC - 1))
                nc.vector.tensor_copy(th[:, ns2], pp[:])

            def conv_ph(nb):
                ns2 = slice(nb * 512, (nb + 1) * 512)
                pp = psum.tile([128, 512], f32, tag="big", name="ph_ps",
                               bufs=2, padded_shape=BIGPAD)
                for ci in range(NCC):
                    nc.tensor.matmul(pp[:], wphd_w[ci], fmb[ci][nb],
                                     start=(ci == 0), stop=(ci == NCC - 1))
                nc.vector.tensor_copy(ph[:, ns2], pp[:])

            def conv_g(nb):
                ns2 = slice(nb * 512, (nb + 1) * 512)
                gp = psum.tile([L, 512], f32, tag="big", name="g_ps",
                               bufs=2, padded_shape=BIGPAD)
                for ci in range(NCC):
                    nc.tensor.matmul(gp[:], wg_w[ci], fmb[ci][nb],
                                     start=(ci == 0), stop=(ci == NCC - 1))
                nc.vector.tensor_copy(g_sb[:, ns2], gp[:])
                for j in range(4):
                    mc = nb * 4 + j
                    tp = psum.tile([128, 64], bf16, tag="tt", name="gtp",
                                   bufs=1, padded_shape=[128, 512])
                    nc.tensor.transpose(tp[:], g_sb[:, mc * 128:(mc + 1) * 128],
                                        id64[:])
                    nc.vector.tensor_copy(gto[:, mc * 65:mc * 65 + 64], tp[:])

            def conv_m2f(nb, pc):
                ns2 = slice(nb * 512, (nb + 1) * 512)
                mp = psum.tile([128, 512], f32, tag="big", name="m2_ps",
                               bufs=2, padded_shape=BIGPAD)
                for ci in range(NCC):
                    nc.tensor.matmul(mp[:], m2_w[pc][ci], fmb[ci][nb],
                                     start=(ci == 0), stop=(ci == NCC - 1))
                nc.vector.tensor_copy(s2f[pc][:, ns2], mp[:])

            # preamble: blocks 0..2 of th/ph/g; the rest are SA fills
            for nb in range(3):
                conv_th(nb)
                conv_ph(nb)
                conv_g(nb)

            # fills for SA(0): ph/g blocks 3..7, >=3 groups ahead of first use
            fills0 = {
                1: [lambda: conv_ph(3), lambda: conv_ph(4)],
                2: [lambda: conv_g(3), lambda: conv_ph(5)],
                3: [lambda: conv_g(4), lambda: conv_ph(6)],
                4: [lambda: conv_g(5), lambda: conv_ph(7)],
                5: [lambda: conv_g(6)],
                6: [lambda: conv_g(7)],
            }
            fill_m2f = [(b, pc) for b in range(NB) for pc in range(2)]

            # ---------------- normalize helpers ----------------
            def norm_step(at, tbt, k):
                tp = psum.tile([128, 65], bf16, tag="tt", name="ntp",
                               bufs=1, padded_shape=[128, 512])
                nc.tensor.transpose(tp[:], at[:, k * 128:(k + 1) * 128],
                                    id65[:])
                rc = sb.tile([128, 1], f32, tag="rc", name="rc", bufs=2)
                nc.vector.reciprocal(rc[:], tp[:, 64:65])
                nc.vector.tensor_scalar_mul(tbt[:, k * 64:(k + 1) * 64],
                                            tp[:, 0:64], rc[:])

            def norm_dma(nbl, tbt, scr, dst_row0):
                scr_view = scr[nbl * 512:(nbl + 1) * 512, :].rearrange(
                    "(k p) c -> p k c", k=4)
                tbt_view = tbt[:].rearrange("p (k c) -> p k c", k=4)
                nc.sync.dma_start(scr_view, tbt_view)
                lat_view = scr[:].rearrange("(a b) c -> a (b c)", a=L)
                nc.sync.dma_start(latcat[dst_row0 + nbl * 8:
                                         dst_row0 + (nbl + 1) * 8, :],
                                  lat_view[nbl * 8:(nbl + 1) * 8, :])

            # ---------------- SA: ACT-paced pipeline ----------------
            sa_norm = {}
            for nb in range(NB):
                ns = slice(nb * 512, (nb + 1) * 512)
                pvp = psum.tile([65, 512], f32, tag="pv", name="pv", bufs=1,
                                padded_shape=[128, 512])
                prev = None
                for gi, grp in enumerate(GROUPS):
                    w = 512 * len(grp)
                    stp = psum.tile([128, w], f32, tag="big", name="st",
                                    bufs=2, padded_shape=BIGPAD)
                    # pair (h0,h64) + single (h0)
                    for j, mc in enumerate(grp[:2]):
                        hp = slice(64 * j, 64 * j + 64)
                        nc.tensor.matmul(
                            stp[:, j * 512:(j + 1) * 512],
                            ph[hp, mc * 128:(mc + 1) * 128],
                            th[hp, ns],
                            start=True, stop=True,
                            tile_position=(64 * j, 0))
                    if len(grp) == 3:
                        mc = grp[2]
                        nc.tensor.matmul(
                            stp[:, 1024:1536],
                            ph[0:64, mc * 128:(mc + 1) * 128],
                            th[0:64, ns],
                            start=True, stop=True,
                            tile_position=(0, 0))
                    ptt = sb.tile([128, w], bf16, tag="pt", name="pt",
                                  bufs=3)
                    nc.scalar.activation(ptt[:], stp[:], AFT.Exp)
                    if nb == 0:
                        for f in fills0.get(gi, []):
                            f()
                    else:
                        if 1 <= gi <= 4:
                            norm_step(*sa_norm[nb - 1], gi - 1)
                        if gi == 5:
                            norm_dma(nb - 1, sa_norm.pop(nb - 1)[1], sc1, 0)
                        if gi == 6 and nb < NB - 1:
                            conv_th(nb + 1)
                        if gi in (7, 9) and fill_m2f:
                            conv_m2f(*fill_m2f.pop(0))
                        if gi == 8 and nb >= 6 and fill_m2f:
                            conv_m2f(*fill_m2f.pop(0))
                    if prev is not None:
                        pgrp, pptt = prev
                        for j, mc in enumerate(pgrp):
                            nc.tensor.matmul(
                                pvp[:], gto[:, mc * 65:(mc + 1) * 65],
                                pptt[:, j * 512:(j + 1) * 512],
                                start=(mc == 0), stop=(mc == NM - 1))
                    prev = (grp, ptt)
                pgrp, pptt = prev
                for j, mc in enumerate(pgrp):
                    nc.tensor.matmul(pvp[:], gto[:, mc * 65:(mc + 1) * 65],
                                     pptt[:, j * 512:(j + 1) * 512],
                                     start=(mc == 0), stop=(mc == NM - 1))
                at = sb.tile([65, 512], bf16, tag="at", name="at", bufs=2)
                nc.vector.tensor_copy(at[:], pvp[:])
                if debug_dump and nb == 0:
                    nc.sync.dma_start(dbg_at, at[:])
                tbt = sb.tile([128, 256], bf16, tag="tb", name="tb", bufs=2)
                sa_norm[nb] = (at, tbt)

            # ---------------- MoCA ----------------
            def s2_fm(nb):
                ns = slice(nb * 512, (nb + 1) * 512)
                s2p = psum.tile([128, 1024], f32, tag="big", name="s2",
                                bufs=2, padded_shape=BIGPAD)
                for pc in range(2):
                    nc.tensor.matmul(s2p[:, pc * 512:(pc + 1) * 512],
                                     id128[:], s2f[pc][:, ns],
                                     start=True, stop=False)
                return s2p

            def s2_lat(nb, s2p):
                ns = slice(nb * 512, (nb + 1) * 512)
                for pc in range(2):
                    nc.tensor.matmul(s2p[:, pc * 512:(pc + 1) * 512],
                                     m2b_w[:, pc * 128:(pc + 1) * 128],
                                     latcat[0:64, ns],
                                     start=False, stop=True)
                p2t = sb.tile([128, 1024], bf16, tag="p2t", name="p2t",
                              bufs=3)
                nc.scalar.activation(p2t[:], s2p[:], AFT.Exp)
                return p2t

            def pv2_step(nb, p2t):
                pvq = psum.tile([65, 512], f32, tag="pv", name="pv2", bufs=1,
                                padded_shape=[128, 512])
                for pc in range(2):
                    nc.tensor.matmul(pvq[:], p2w[:, pc * 65:(pc + 1) * 65],
                                     p2t[:, pc * 512:(pc + 1) * 512],
                                     start=(pc == 0), stop=(pc == 1))
                at2 = sb.tile([65, 512], bf16, tag="at", name="at2", bufs=2)
                nc.vector.tensor_copy(at2[:], pvq[:])
                tb2 = sb.tile([128, 256], bf16, tag="tb", name="tb2", bufs=2)
                return (at2, tb2)

            # trailing SA norms interleaved with the MoCA fm-score prologue
            s2ps = {0: s2_fm(0)}
            norm_step(*sa_norm[NB - 1], 0)
            s2ps[1] = s2_fm(1)
            for k in range(1, 4):
                norm_step(*sa_norm[NB - 1], k)
            norm_dma(NB - 1, sa_norm.pop(NB - 1)[1], sc1, 0)

            moca = {}
            mo_at = {}
            for nb in range(NB):
                if nb >= 2:
                    norm_step(*mo_at[nb - 2], 2)
                moca[nb] = s2_lat(nb, s2ps.pop(nb))
                if nb >= 2:
                    norm_step(*mo_at[nb - 2], 3)
                if nb >= 1:
                    mo_at[nb - 1] = pv2_step(nb - 1, moca.pop(nb - 1))
                if nb >= 2:
                    norm_dma(nb - 2, mo_at.pop(nb - 2)[1], sc2, 64)
                if nb + 2 < NB:
                    s2ps[nb + 2] = s2_fm(nb + 2)
                if nb >= 1:
                    norm_step(*mo_at[nb - 1], 0)
                    norm_step(*mo_at[nb - 1], 1)
            norm_step(*mo_at[NB - 2], 2)
            norm_step(*mo_at[NB - 2], 3)
            mo_at[NB - 1] = pv2_step(NB - 1, moca.pop(NB - 1))
            norm_dma(NB - 2, mo_at.pop(NB - 2)[1], sc2, 64)
            for k in range(4):
                norm_step(*mo_at[NB - 1], k)
            norm_dma(NB - 1, mo_at.pop(NB - 1)[1], sc2, 64)

            # ------------- tail: out = [wosa|womo]@[lat;lat2] + fm ----------
            for nb in range(NB):
                ns = slice(nb * 512, (nb + 1) * 512)
                ob = sb.tile([128, 2048], bf16, tag="ob", name="ob",
                             bufs=2)
                for g2 in range(2):
                    oc = psum.tile([128, 1024], f32, tag="big", name="oc",
                                   bufs=2, padded_shape=BIGPAD)
                    for h in range(2):
                        cc = g2 * 2 + h
                        nc.tensor.matmul(oc[:, h * 512:(h + 1) * 512],
                                         wcat_w[:, cc * 128:(cc + 1) * 128],
                                         latcat[:, ns],
                                         start=True, stop=True)
                        if h == 0:
                            nc.vector.tensor_add(
                                ob[:, cc * 512:(cc + 1) * 512],
                                oc[:, h * 512:(h + 1) * 512],
                                fmb[cc][nb])
                        else:
                            tmp = sb.tile([128, 512], bf16, tag="rtmp",
                                          name="rtmp", bufs=3)
                            nc.scalar.activation(tmp[:],
                                                 oc[:, h * 512:(h + 1) * 512],
                                                 AFT.Copy)
                            nc.gpsimd.tensor_add(
                                ob[:, cc * 512:(cc + 1) * 512], tmp[:],
                                fmb[cc][nb])
                ov = out_d[:, ns].rearrange("(u p) c -> p u c", u=NCC)
                ob_view = ob[:].rearrange("p (u c) -> p u c", u=NCC)
                nc.sync.dma_start(ov, ob_view)
            if debug_dump:
                nc.sync.dma_start(dbg_th, th[:])
                nc.sync.dma_start(dbg_ph, ph[:])
                nc.sync.dma_start(dbg_g, g_sb[:])
                nc.sync.dma_start(dbg_lat, latcat[:])

    nc.compile()
    return nc


def _get_runner(reps=1):
    """Build the Bass program once and return a cached jitted SPMD callable."""
    key = ("runner", reps)
    if key in _STATE:
        return _STATE[key]

    import jax
    import numpy as np
    from jax.experimental.shard_map import shard_map
    from jax.sharding import Mesh, PartitionSpec
    import concourse.mybir as mybir
    from concourse import bass2jax

    nc = _build_program(reps=reps)
    bass2jax.install_neuronx_cc_hook()

    partition_name = (nc.partition_id_tensor.name
                      if nc.partition_id_tensor else None)
    in_names, out_names, out_avals, zero_shapes = [], [], [], []
    for alloc in nc.m.functions[0].allocations:
        if not isinstance(alloc, mybir.MemoryLocationSet):
            continue
        name = alloc.memorylocations[0].name
        if alloc.kind == "ExternalInput":
            if name != partition_name:
                in_names.append(name)
        elif alloc.kind == "ExternalOutput":
            out_names.append(name)
            shape = tuple(alloc.tensor_shape)
            dtype = mybir.dt.np(alloc.dtype)
            out_avals.append(jax.core.ShapedArray(shape, dtype))
            zero_shapes.append((shape, dtype))
    n_params = len(in_names)
    all_in_names = list(in_names) + list(out_names)
    if partition_name is not None:
        all_in_names.append(partition_name)

    def _body(*args):
        operands = list(args)
        if partition_name is not None:
            operands.append(bass2jax.partition_id_tensor())
        outs = bass2jax._bass_exec_p.bind(
            *operands,
            out_avals=tuple(out_avals),
            in_names=tuple(all_in_names),
            out_names=tuple(out_names),
            lowering_input_output_aliases=(),
            sim_require_finite=True,
            sim_require_nnan=True,
            nc=nc,
        )
        return tuple(outs)

    devices = jax.devices()[:N_CORES]
    mesh = Mesh(np.asarray(devices), ("core",))
    n_outs = len(out_names)
    donate = tuple(range(n_params, n_params + n_outs))
    sharded = jax.jit(
        shard_map(_body, mesh=mesh,
                  in_specs=(PartitionSpec("core"),) * (n_params + n_outs),
                  out_specs=(PartitionSpec("core"),) * n_outs,
                  check_rep=False),
        donate_argnums=donate, keep_unused=True)

    runner = {
        "nc": nc, "sharded": sharded, "in_names": in_names,
        "out_names": out_names, "zero_shapes": zero_shapes,
        "n_params": n_params,
    }
    _STATE[key] = runner
    return runner


def _prep_in_maps(feature_map, concepts, w_theta, w_phi, w_g, w_o,
                  gamma_sa, gamma_moca):
    import ml_dtypes

    bf16 = ml_dtypes.bfloat16

    feature_map = np.asarray(feature_map, dtype=np.float32)
    concepts = np.asarray(concepts, dtype=np.float32)
    w_theta = np.asarray(w_theta, dtype=np.float32)
    w_phi = np.asarray(w_phi, dtype=np.float32)
    w_g = np.asarray(w_g, dtype=np.float32)
    w_o = np.asarray(w_o, dtype=np.float32)
    gamma_sa = np.float32(gamma_sa)
    gamma_moca = np.float32(gamma_moca)

    gain = np.float32(1.0 / np.sqrt(C))
    gain_o = np.float32(1.0 / np.sqrt(L))

    wth_t = w_theta.T * gain                                 # [C, L]
    wph_t = w_phi.T * gain
    wthdup = np.concatenate([wth_t, wth_t], axis=1)          # [C, 128]
    wphdup = np.concatenate([wph_t, wph_t], axis=1)
    wg_t = w_g.T * gain                                      # [C, L]
    m2 = concepts @ (w_theta * gain)                         # [P, C]
    m2t = m2.T                                               # [C, P]
    m2b = (gamma_sa * gain_o) * (m2 @ w_o)                   # [P, L]
    m2bt = m2b.T                                             # [L, P]
    wosa = w_o.T * (gain_o * gamma_sa)                       # [L, C]
    womo = w_o.T * (gain_o * gamma_moca)
    wcat = np.concatenate([wosa, womo], axis=0)              # [128, C]

    # pack all weights into one [128, 3200] wall (layout mirrors kernel APs)
    wall = np.zeros((128, 3200), np.float32)
    for ci in range(4):
        cs = slice(ci * 128, (ci + 1) * 128)
        wall[:, ci * 128:(ci + 1) * 128] = wthdup[cs, :]
        wall[:, 512 + ci * 128:512 + (ci + 1) * 128] = wphdup[cs, :]
        wall[:, 1024 + ci * 64:1024 + (ci + 1) * 64] = wg_t[cs, :]
        for pc in range(2):
            wall[:, 1280 + pc * 512 + ci * 128:
                 1280 + pc * 512 + (ci + 1) * 128] = m2t[cs,
                                                         pc * 128:(pc + 1) * 128]
    wall[0:64, 2304:2560] = m2bt
    wall[:, 2560:3072] = wcat
    wall[:, 3072:3136] = concepts[0:128, :]
    wall[:, 3136:3200] = concepts[128:256, :]
    wall = np.ascontiguousarray(wall).astype(bf16)
    fm_flat = feature_map.reshape(B, C, HW)

    in_maps = []
    for b in range(N_CORES):
        in_maps.append({
            "fmb": np.ascontiguousarray(fm_flat[b]).astype(bf16),
            "wall": wall,
        })
    return in_maps


def _run(in_maps):
    r = _get_runner()
    n_params = r["n_params"]
    concat_in = [
        np.concatenate([np.asarray(in_maps[c][name])
                        for c in range(N_CORES)], axis=0)
        for name in r["in_names"]
    ]
    concat_zeros = [np.zeros((N_CORES * s[0], *s[1:]), d)
                    for (s, d) in r["zero_shapes"]]
    out_arrs = r["sharded"](*concat_in, *concat_zeros)
    per_core = []
    for c in range(N_CORES):
        per_core.append({
            name: np.asarray(out_arrs[i]).reshape(
                N_CORES, *r["zero_shapes"][i][0])[c]
            for i, name in enumerate(r["out_names"])
        })
    return per_core


def kernel(feature_map, concepts, w_theta, w_phi, w_g, w_o,
           gamma_sa, gamma_moca):
    in_maps = _prep_in_maps(feature_map, concepts, w_theta, w_phi, w_g, w_o,
                            gamma_sa, gamma_moca)
    per_core = _run(in_maps)
    out = np.stack([per_core[b]["out"].astype(np.float32).reshape(C, H, W)
                    for b in range(B)], axis=0)
    return out.astype(np.float32)


# revision 16
# speedup vs baseline: 1.3691x; 1.0387x over previous
"""Trainium2 Bass kernel for nn_MoCA (self-attention + momentum concept attention).

Sharding: pure data parallel - batch dim (B=8) sharded 1 batch per NeuronCore,
weights/concept pool replicated. No collectives.

v6 - ACT(exp)-roofline-targeted redesign (all bf16 compute):
  * ST keeps the baseline quadrant pairing (dup'd th/ph convs, tile_position)
    -- required to sustain full PE clock -- but exp tiles are widened to
    [128,1536] (3 m-chunks: one pair + one single ST per group): 11 ACT
    instructions per n-block instead of 16.
  * all large DMAs are split into <=128KB chains: per-ring DMA bandwidth is
    ~23GB/s, so a single 512KB chain takes 22us. Splitting gets the first
    conv inputs on-chip in ~5us instead of 22us.
  * fm enters as host-cast bf16 (residual + convs); output leaves as bf16
    and is upcast on host. Halves the fm/out DMA vs f32.
  * conv work (ph/g for all blocks, th for block nb+1, M2@fm halves) is
    scheduled as fills inside the SA loop at >=3 exp-groups before first
    consumption (closer than that races the LDWEIGHTS prefetch).
  * MoCA scores: s2 = I@s2f + m2b@lat accumulated in PSUM; the identity
    matmuls are emitted before lat is ready to fill the SA->MoCA bubble.
  * norm path (PE transpose + reciprocal scale + DRAM reinterpret roundtrip)
    in bf16, spread across exp groups.
  * PSUM: big [128,1536] x2 (6 banks) + pv x1 + tt x1 = 8 banks.
"""
import sys

if '/opt/trn_rl_repo' not in sys.path:
    sys.path.insert(0, '/opt/trn_rl_repo')

import numpy as np

C, L, H, W, P = 512, 64, 64, 64, 256
HW = H * W
B = 8
N_CORES = 8

_STATE: dict = {}


def _build_program(reps=1, num_devices=N_CORES, debug_dump=False):
    import concourse.bass as bass
    import concourse.bacc as bacc
    import concourse.mybir as mybir
    from concourse import tile
    from concourse.masks import make_identity

    dt = mybir.dt
    AFT = mybir.ActivationFunctionType
    f32, bf16 = dt.float32, dt.bfloat16

    nc = bacc.Bacc("TRN2", target_bir_lowering=False, debug=False,
                   enable_asserts=False, num_devices=num_devices)

    fmb_d = nc.dram_tensor("fmb", [C, HW], bf16, kind="ExternalInput").ap()
    wall_d = nc.dram_tensor("wall", [128, 3200], bf16, kind="ExternalInput").ap()
    out_d = nc.dram_tensor("out", [C, HW], bf16, kind="ExternalOutput").ap()
    if debug_dump:
        dbg_th = nc.dram_tensor("dbg_th", [128, HW], bf16, kind="ExternalOutput").ap()
        dbg_ph = nc.dram_tensor("dbg_ph", [128, HW], bf16, kind="ExternalOutput").ap()
        dbg_g = nc.dram_tensor("dbg_g", [L, HW], bf16, kind="ExternalOutput").ap()
        dbg_lat = nc.dram_tensor("dbg_lat", [128, HW], bf16, kind="ExternalOutput").ap()
        dbg_at = nc.dram_tensor("dbg_at", [65, 512], bf16, kind="ExternalOutput").ap()

    NB = HW // 512          # 8 n-blocks of 512
    NM = HW // 128          # 32 m-chunks of 128
    NCC = C // 128          # 4 channel chunks
    # m-chunk groups per n-block: 10x3 + 1x2 (one exp instruction each)
    GROUPS = [tuple(range(3 * i, 3 * i + 3)) for i in range(10)] + [(30, 31)]
    BIGPAD = [128, 1536]

    with tile.TileContext(nc) as tc:
      for _rep in range(reps):
        with tc.tile_pool(name="sb", bufs=1) as sb, \
             tc.tile_pool(name="dram", bufs=1, space="DRAM") as dp, \
             tc.tile_pool(name="ps", bufs=1, space="PSUM") as psum:

            sc1 = dp.tile([HW, L], bf16, tag="sc1", name="sc1")
            sc2 = dp.tile([HW, L], bf16, tag="sc2", name="sc2")

            # ---------------- persistent tiles ----------------
            fmball = sb.tile([128, NCC * HW], bf16, tag="fmball",
                             name="fmball")
            fmb = [[fmball[:, ci * HW + nb * 512:ci * HW + (nb + 1) * 512]
                    for nb in range(NB)] for ci in range(NCC)]
            th = sb.tile([128, HW], bf16, tag="th", name="th")
            ph = sb.tile([128, HW], bf16, tag="ph", name="ph")
            g_sb = sb.tile([L, HW], bf16, tag="g_sb", name="g_sb")
            gto = sb.tile([128, NM * 65], bf16, tag="gto", name="gto")
            lat1 = sb.tile([64, HW], bf16, tag="lat1", name="lat1")
            latmo = sb.tile([128, HW], bf16, tag="latmo", name="latmo")

            wall_w = sb.tile([128, 3200], bf16, tag="wall", name="wall")
            wthd_w = [wall_w[:, i * 128:(i + 1) * 128] for i in range(NCC)]
            wphd_w = [wall_w[:, 512 + i * 128:512 + (i + 1) * 128]
                      for i in range(NCC)]
            wg_w = [wall_w[:, 1024 + i * 64:1024 + (i + 1) * 64]
                    for i in range(NCC)]
            m2_w = [[wall_w[:, 1280 + pc * 512 + i * 128:
                            1280 + pc * 512 + (i + 1) * 128]
                     for i in range(NCC)] for pc in range(2)]
            m2b_w = wall_w[0:64, 2304:2560]
            wcat_w = wall_w[:, 2560:3072]
            p2w = sb.tile([128, 130], bf16, tag="p2w", name="p2w")
            id64 = sb.tile([64, 64], bf16, tag="id64", name="id64")
            id65 = sb.tile([65, 65], bf16, tag="id65", name="id65")
            id128 = sb.tile([128, 128], bf16, tag="id128", name="id128")

            make_identity(nc, id64[:])
            make_identity(nc, id65[:])
            make_identity(nc, id128[:])
            nc.vector.memset(p2w[:], 1.0)
            nc.vector.memset(gto[:], 1.0)

            # one DMA for all weights; 8 block-DMAs for fm (hardware
            # packetizes each instruction across 16 engines at 256B)
            nc.sync.dma_start(wall_w[:], wall_d)
            fm_dst = fmball[:].rearrange("p (ci n) -> p ci n", ci=NCC)
            fm_src = fmb_d[:].rearrange("(ci p) n -> p ci n", ci=NCC)
            for nb in range(NB):
                ns = slice(nb * 512, (nb + 1) * 512)
                nc.sync.dma_start(fm_dst[:, :, ns], fm_src[:, :, ns])
            nc.vector.tensor_copy(p2w[:, 0:64], wall_w[:, 3072:3136])
            nc.vector.tensor_copy(p2w[:, 65:129], wall_w[:, 3136:3200])

            # ---------------- conv helpers ----------------
            def conv_th(nb):
                ns2 = slice(nb * 512, (nb + 1) * 512)
                pp = psum.tile([128, 512], f32, tag="big", name="th_ps",
                               bufs=2, padded_shape=BIGPAD)
                for ci in range(NCC):
                    nc.tensor.matmul(pp[:], wthd_w[ci], fmb[ci][nb],
                                     start=(ci == 0), stop=(ci == NCC - 1))
                nc.vector.tensor_copy(th[:, ns2], pp[:])

            def conv_ph(nb):
                ns2 = slice(nb * 512, (nb + 1) * 512)
                pp = psum.tile([128, 512], f32, tag="big", name="ph_ps",
                               bufs=2, padded_shape=BIGPAD)
                for ci in range(NCC):
                    nc.tensor.matmul(pp[:], wphd_w[ci], fmb[ci][nb],
                                     start=(ci == 0), stop=(ci == NCC - 1))
                nc.vector.tensor_copy(ph[:, ns2], pp[:])

            def conv_g(nb):
                ns2 = slice(nb * 512, (nb + 1) * 512)
                gp = psum.tile([L, 512], f32, tag="big", name="g_ps",
                               bufs=2, padded_shape=BIGPAD)
                for ci in range(NCC):
                    nc.tensor.matmul(gp[:], wg_w[ci], fmb[ci][nb],
                                     start=(ci == 0), stop=(ci == NCC - 1))
                nc.vector.tensor_copy(g_sb[:, ns2], gp[:])
                for j in range(4):
                    mc = nb * 4 + j
                    tp = psum.tile([128, 64], bf16, tag="tt", name="gtp",
                                   bufs=1, padded_shape=[128, 512])
                    nc.tensor.transpose(tp[:], g_sb[:, mc * 128:(mc + 1) * 128],
                                        id64[:])
                    nc.vector.tensor_copy(gto[:, mc * 65:mc * 65 + 64], tp[:])

            # PE warmup: keep the array streaming while the fm DMA lands so
            # the clock is at full p-state when the real convs start
            import os as _os
            if _os.environ.get("NOWARM") != "1":
                wu = psum.tile([128, 128], f32, tag="pv", name="wu", bufs=1,
                               padded_shape=[128, 512])
                for _ in range(48):
                    nc.tensor.matmul(wu[:], id128[:], id128[:],
                                     start=True, stop=True)

            # preamble: th(0), ph(0..2), g(0..2); the rest are SA fills
            conv_th(0)
            for nb in range(3):
                conv_ph(nb)
                conv_g(nb)

            # fills for SA(0): ph/g blocks 3..7, >=3 groups ahead of first use
            fills0 = {
                1: [lambda: conv_ph(3), lambda: conv_ph(4)],
                2: [lambda: conv_g(3), lambda: conv_ph(5)],
                3: [lambda: conv_g(4), lambda: conv_ph(6)],
                4: [lambda: conv_g(5), lambda: conv_ph(7)],
                5: [lambda: conv_g(6)],
                6: [lambda: conv_g(7), lambda: conv_th(1)],
                7: [lambda: conv_th(2)],
            }

            # ---------------- normalize helpers ----------------
            def norm_step(at, tbt, k):
                tp = psum.tile([128, 65], bf16, tag="tt", name="ntp",
                               bufs=1, padded_shape=[128, 512])
                nc.tensor.transpose(tp[:], at[:, k * 128:(k + 1) * 128],
                                    id65[:])
                rc = sb.tile([128, 1], f32, tag="rc", name="rc", bufs=2)
                nc.vector.reciprocal(rc[:], tp[:, 64:65])
                nc.vector.tensor_scalar_mul(tbt[:, k * 64:(k + 1) * 64],
                                            tp[:, 0:64], rc[:])

            def norm_dma(nbl, tbt, scr, dst, dst_row0):
                scr_view = scr[nbl * 512:(nbl + 1) * 512, :].rearrange(
                    "(k p) c -> p k c", k=4)
                tbt_view = tbt[:].rearrange("p (k c) -> p k c", k=4)
                nc.sync.dma_start(scr_view, tbt_view)
                lat_view = scr[:].rearrange("(a b) c -> a (b c)", a=L)
                nc.sync.dma_start(dst[dst_row0 + nbl * 8:
                                      dst_row0 + (nbl + 1) * 8, :],
                                  lat_view[nbl * 8:(nbl + 1) * 8, :])

            # ---------------- SA: ACT-paced pipeline ----------------
            sa_norm = {}
            for nb in range(NB):
                ns = slice(nb * 512, (nb + 1) * 512)
                pvp = psum.tile([65, 512], f32, tag="pv", name="pv", bufs=1,
                                padded_shape=[128, 512])
                prev = None
                for gi, grp in enumerate(GROUPS):
                    w = 512 * len(grp)
                    stp = psum.tile([128, w], f32, tag="big", name="st",
                                    bufs=2, padded_shape=BIGPAD)
                    # pair (h0,h64) + single (h0)
                    for j, mc in enumerate(grp[:2]):
                        hp = slice(64 * j, 64 * j + 64)
                        nc.tensor.matmul(
                            stp[:, j * 512:(j + 1) * 512],
                            ph[hp, mc * 128:(mc + 1) * 128],
                            th[hp, ns],
                            start=True, stop=True,
                            tile_position=(64 * j, 0))
                    if len(grp) == 3:
                        mc = grp[2]
                        nc.tensor.matmul(
                            stp[:, 1024:1536],
                            ph[0:64, mc * 128:(mc + 1) * 128],
                            th[0:64, ns],
                            start=True, stop=True,
                            tile_position=(0, 0))
                    ptt = sb.tile([128, w], bf16, tag="pt", name="pt",
                                  bufs=3)
                    nc.scalar.activation(ptt[:], stp[:], AFT.Exp)
                    if nb == 0:
                        for f in fills0.get(gi, []):
                            f()
                    else:
                        if 1 <= gi <= 4:
                            norm_step(*sa_norm[nb - 1], gi - 1)
                        if gi == 5:
                            norm_dma(nb - 1, sa_norm.pop(nb - 1)[1], sc1, lat1, 0)
                        if gi == 6 and nb < NB - 2:
                            conv_th(nb + 2)
                    if prev is not None:
                        pgrp, pptt = prev
                        for j, mc in enumerate(pgrp):
                            nc.tensor.matmul(
                                pvp[:], gto[:, mc * 65:(mc + 1) * 65],
                                pptt[:, j * 512:(j + 1) * 512],
                                start=(mc == 0), stop=(mc == NM - 1))
                    prev = (grp, ptt)
                pgrp, pptt = prev
                for j, mc in enumerate(pgrp):
                    nc.tensor.matmul(pvp[:], gto[:, mc * 65:(mc + 1) * 65],
                                     pptt[:, j * 512:(j + 1) * 512],
                                     start=(mc == 0), stop=(mc == NM - 1))
                at = sb.tile([65, 512], bf16, tag="at", name="at", bufs=2)
                nc.vector.tensor_copy(at[:], pvp[:])
                if debug_dump and nb == 0:
                    nc.sync.dma_start(dbg_at, at[:])
                tbt = sb.tile([128, 256], bf16, tag="tb", name="tb", bufs=2)
                sa_norm[nb] = (at, tbt)

            # ---------------- MoCA ----------------
            def s2_fm(nb):
                ns = slice(nb * 512, (nb + 1) * 512)
                s2p = psum.tile([128, 1024], f32, tag="big", name="s2",
                                bufs=2, padded_shape=BIGPAD)
                for pc in range(2):
                    for ci in range(NCC):
                        nc.tensor.matmul(s2p[:, pc * 512:(pc + 1) * 512],
                                         m2_w[pc][ci], fmb[ci][nb],
                                         start=(ci == 0), stop=False)
                return s2p

            def s2_lat(nb, s2p):
                ns = slice(nb * 512, (nb + 1) * 512)
                for pc in range(2):
                    nc.tensor.matmul(s2p[:, pc * 512:(pc + 1) * 512],
                                     m2b_w[:, pc * 128:(pc + 1) * 128],
                                     lat1[:, ns],
                                     start=False, stop=True)
                p2t = sb.tile([128, 1024], bf16, tag="p2t", name="p2t",
                              bufs=3)
                nc.scalar.activation(p2t[:], s2p[:], AFT.Exp)
                return p2t

            def pv2_step(nb, p2t):
                pvq = psum.tile([65, 512], f32, tag="pv", name="pv2", bufs=1,
                                padded_shape=[128, 512])
                for pc in range(2):
                    nc.tensor.matmul(pvq[:], p2w[:, pc * 65:(pc + 1) * 65],
                                     p2t[:, pc * 512:(pc + 1) * 512],
                                     start=(pc == 0), stop=(pc == 1))
                at2 = sb.tile([65, 512], bf16, tag="at", name="at2", bufs=2)
                nc.vector.tensor_copy(at2[:], pvq[:])
                tb2 = sb.tile([128, 256], bf16, tag="tb", name="tb2", bufs=2)
                return (at2, tb2)

            # trailing SA norms interleaved with the MoCA fm-score prologue
            s2ps = {0: s2_fm(0)}
            norm_step(*sa_norm[NB - 1], 0)
            s2ps[1] = s2_fm(1)
            for k in range(1, 4):
                norm_step(*sa_norm[NB - 1], k)
            norm_dma(NB - 1, sa_norm.pop(NB - 1)[1], sc1, lat1, 0)

            moca = {}
            mo_at = {}
            for nb in range(NB):
                if nb >= 2:
                    norm_step(*mo_at[nb - 2], 2)
                moca[nb] = s2_lat(nb, s2ps.pop(nb))
                if nb >= 2:
                    norm_step(*mo_at[nb - 2], 3)
                if nb >= 1:
                    mo_at[nb - 1] = pv2_step(nb - 1, moca.pop(nb - 1))
                if nb >= 2:
                    norm_dma(nb - 2, mo_at.pop(nb - 2)[1], sc2, latmo, 64)
                if nb + 2 < NB:
                    s2ps[nb + 2] = s2_fm(nb + 2)
                if nb >= 1:
                    norm_step(*mo_at[nb - 1], 0)
                    norm_step(*mo_at[nb - 1], 1)
            norm_step(*mo_at[NB - 2], 2)
            norm_step(*mo_at[NB - 2], 3)
            mo_at[NB - 1] = pv2_step(NB - 1, moca.pop(NB - 1))
            norm_dma(NB - 2, mo_at.pop(NB - 2)[1], sc2, latmo, 64)
            for k in range(4):
                norm_step(*mo_at[NB - 1], k)
            norm_dma(NB - 1, mo_at.pop(NB - 1)[1], sc2, latmo, 64)

            # ------------- tail: out = [wosa|womo]@[lat;lat2] + fm ----------
            fm_v = fmball[:].rearrange("p (ci n) -> p ci n", ci=NCC)
            for k in range(8):
                nc.vector.tensor_copy(latmo[0:64, k * 512:(k + 1) * 512],
                                      lat1[:, k * 512:(k + 1) * 512])
            for nb in range(NB):
                ns = slice(nb * 512, (nb + 1) * 512)
                ob = sb.tile([128, 2048], bf16, tag="ob", name="ob",
                             bufs=2)
                for g2 in range(2):
                    oc = psum.tile([128, 1024], f32, tag="big", name="oc",
                                   bufs=2, padded_shape=BIGPAD)
                    for h in range(2):
                        cc = g2 * 2 + h
                        nc.tensor.matmul(oc[:, h * 512:(h + 1) * 512],
                                         wcat_w[:, cc * 128:(cc + 1) * 128],
                                         latmo[:, ns],
                                         start=True, stop=True)
                    fm2 = fm_v[:, 2 * g2:2 * g2 + 2, ns]
                    obv = ob[:].rearrange("p (u c) -> p u c", u=2)[
                        :, 2 * g2:2 * g2 + 2 if False else slice(None), :]                         if False else ob[:, g2 * 1024:(g2 + 1) * 1024].rearrange(
                            "p (u c) -> p u c", u=2)
                    ocv = oc[:].rearrange("p (u c) -> p u c", u=2)
                    if (nb * 2 + g2) % 3 == 2:
                        tmp = sb.tile([128, 1024], bf16, tag="rtmp",
                                      name="rtmp", bufs=2)
                        nc.scalar.activation(tmp[:], oc[:], AFT.Copy)
                        for h in range(2):
                            cc = g2 * 2 + h
                            nc.gpsimd.tensor_add(
                                ob[:, cc * 512:(cc + 1) * 512],
                                tmp[:, h * 512:(h + 1) * 512],
                                fmb[cc][nb])
                    else:
                        nc.vector.tensor_add(obv, ocv, fm2)
                ov = out_d[:, ns].rearrange("(u p) c -> p u c", u=NCC)
                ob_view = ob[:].rearrange("p (u c) -> p u c", u=NCC)
                nc.sync.dma_start(ov, ob_view)
            if debug_dump:
                nc.sync.dma_start(dbg_th, th[:])
                nc.sync.dma_start(dbg_ph, ph[:])
                nc.sync.dma_start(dbg_g, g_sb[:])
                nc.sync.dma_start(dbg_lat[0:64, :], lat1[:])
                nc.sync.dma_start(dbg_lat[64:128, :], latmo[64:128, :])

    nc.compile()
    return nc


def _get_runner(reps=1):
    """Build the Bass program once and return a cached jitted SPMD callable."""
    key = ("runner", reps)
    if key in _STATE:
        return _STATE[key]

    import jax
    import numpy as np
    from jax.experimental.shard_map import shard_map
    from jax.sharding import Mesh, PartitionSpec
    import concourse.mybir as mybir
    from concourse import bass2jax

    nc = _build_program(reps=reps)
    bass2jax.install_neuronx_cc_hook()

    partition_name = (nc.partition_id_tensor.name
                      if nc.partition_id_tensor else None)
    in_names, out_names, out_avals, zero_shapes = [], [], [], []
    for alloc in nc.m.functions[0].allocations:
        if not isinstance(alloc, mybir.MemoryLocationSet):
            continue
        name = alloc.memorylocations[0].name
        if alloc.kind == "ExternalInput":
            if name != partition_name:
                in_names.append(name)
        elif alloc.kind == "ExternalOutput":
            out_names.append(name)
            shape = tuple(alloc.tensor_shape)
            dtype = mybir.dt.np(alloc.dtype)
            out_avals.append(jax.core.ShapedArray(shape, dtype))
            zero_shapes.append((shape, dtype))
    n_params = len(in_names)
    all_in_names = list(in_names) + list(out_names)
    if partition_name is not None:
        all_in_names.append(partition_name)

    def _body(*args):
        operands = list(args)
        if partition_name is not None:
            operands.append(bass2jax.partition_id_tensor())
        outs = bass2jax._bass_exec_p.bind(
            *operands,
            out_avals=tuple(out_avals),
            in_names=tuple(all_in_names),
            out_names=tuple(out_names),
            lowering_input_output_aliases=(),
            sim_require_finite=True,
            sim_require_nnan=True,
            nc=nc,
        )
        return tuple(outs)

    devices = jax.devices()[:N_CORES]
    mesh = Mesh(np.asarray(devices), ("core",))
    n_outs = len(out_names)
    donate = tuple(range(n_params, n_params + n_outs))
    sharded = jax.jit(
        shard_map(_body, mesh=mesh,
                  in_specs=(PartitionSpec("core"),) * (n_params + n_outs),
                  out_specs=(PartitionSpec("core"),) * n_outs,
                  check_rep=False),
        donate_argnums=donate, keep_unused=True)

    runner = {
        "nc": nc, "sharded": sharded, "in_names": in_names,
        "out_names": out_names, "zero_shapes": zero_shapes,
        "n_params": n_params,
    }
    _STATE[key] = runner
    return runner


def _prep_in_maps(feature_map, concepts, w_theta, w_phi, w_g, w_o,
                  gamma_sa, gamma_moca):
    import ml_dtypes

    bf16 = ml_dtypes.bfloat16

    feature_map = np.asarray(feature_map, dtype=np.float32)
    concepts = np.asarray(concepts, dtype=np.float32)
    w_theta = np.asarray(w_theta, dtype=np.float32)
    w_phi = np.asarray(w_phi, dtype=np.float32)
    w_g = np.asarray(w_g, dtype=np.float32)
    w_o = np.asarray(w_o, dtype=np.float32)
    gamma_sa = np.float32(gamma_sa)
    gamma_moca = np.float32(gamma_moca)

    gain = np.float32(1.0 / np.sqrt(C))
    gain_o = np.float32(1.0 / np.sqrt(L))

    wth_t = w_theta.T * gain                                 # [C, L]
    wph_t = w_phi.T * gain
    wthdup = np.concatenate([wth_t, wth_t], axis=1)          # [C, 128]
    wphdup = np.concatenate([wph_t, wph_t], axis=1)
    wg_t = w_g.T * gain                                      # [C, L]
    m2 = concepts @ (w_theta * gain)                         # [P, C]
    m2t = m2.T                                               # [C, P]
    m2b = (gamma_sa * gain_o) * (m2 @ w_o)                   # [P, L]
    m2bt = m2b.T                                             # [L, P]
    wosa = w_o.T * (gain_o * gamma_sa)                       # [L, C]
    womo = w_o.T * (gain_o * gamma_moca)
    wcat = np.concatenate([wosa, womo], axis=0)              # [128, C]

    # pack all weights into one [128, 3200] wall (layout mirrors kernel APs)
    wall = np.zeros((128, 3200), np.float32)
    for ci in range(4):
        cs = slice(ci * 128, (ci + 1) * 128)
        wall[:, ci * 128:(ci + 1) * 128] = wthdup[cs, :]
        wall[:, 512 + ci * 128:512 + (ci + 1) * 128] = wphdup[cs, :]
        wall[:, 1024 + ci * 64:1024 + (ci + 1) * 64] = wg_t[cs, :]
        for pc in range(2):
            wall[:, 1280 + pc * 512 + ci * 128:
                 1280 + pc * 512 + (ci + 1) * 128] = m2t[cs,
                                                         pc * 128:(pc + 1) * 128]
    wall[0:64, 2304:2560] = m2bt
    wall[:, 2560:3072] = wcat
    wall[:, 3072:3136] = concepts[0:128, :]
    wall[:, 3136:3200] = concepts[128:256, :]
    wall = np.ascontiguousarray(wall).astype(bf16)
    fm_flat = feature_map.reshape(B, C, HW)

    in_maps = []
    for b in range(N_CORES):
        in_maps.append({
            "fmb": np.ascontiguousarray(fm_flat[b]).astype(bf16),
            "wall": wall,
        })
    return in_maps


def _run(in_maps):
    r = _get_runner()
    n_params = r["n_params"]
    concat_in = [
        np.concatenate([np.asarray(in_maps[c][name])
                        for c in range(N_CORES)], axis=0)
        for name in r["in_names"]
    ]
    concat_zeros = [np.zeros((N_CORES * s[0], *s[1:]), d)
                    for (s, d) in r["zero_shapes"]]
    out_arrs = r["sharded"](*concat_in, *concat_zeros)
    per_core = []
    for c in range(N_CORES):
        per_core.append({
            name: np.asarray(out_arrs[i]).reshape(
                N_CORES, *r["zero_shapes"][i][0])[c]
            for i, name in enumerate(r["out_names"])
        })
    return per_core


def kernel(feature_map, concepts, w_theta, w_phi, w_g, w_o,
           gamma_sa, gamma_moca):
    in_maps = _prep_in_maps(feature_map, concepts, w_theta, w_phi, w_g, w_o,
                            gamma_sa, gamma_moca)
    per_core = _run(in_maps)
    out = np.stack([per_core[b]["out"].astype(np.float32).reshape(C, H, W)
                    for b in range(B)], axis=0)
    return out.astype(np.float32)
